# revision 22
# baseline (speedup 1.0000x reference)
"""Multi-head causal attention (B=4, S=2048, E=1024, H=16, Dh=64) on 8
Trainium2 NeuronCores.

Sharding: data-parallel over the 4 batch elements x tensor-parallel over
heads (2 groups of 8). Core 2b+g handles batch b, heads 8g..8g+7. Each core
computes Q^T/K^T (head dim on partitions), V (natural layout, with a fused
ones-column so the attention-weight row sums fall out of the same matmul),
block-causal scores in transposed [kv, q] layout (no transposes needed:
softmax normalization is a reciprocal + selector-matmul partition
broadcast), the local-head context, and the output projection against its
slice of Wo. The two partial projections per batch are summed on the host
(the TP "all-reduce" of the sharding hint, done at gather time), which also
absorbs the out-transpose: the kernel emits out^T [E, S] in bf16.

All matmuls run in bf16 (fp32 accumulate): f32r streams at the same 1
cycle/row but draws enough power that the core throttles to a ~50%
utilization cap for half the runtime; bf16 also trims the r=3 diagonal
scores block to its live 128 columns (no N>=256 full-rate requirement).

KEY SCHEDULE IDEA (from per-engine totals of the unfused ancestors): the
QKV projections (stage A) are tensor-bound with an idle scalar engine,
while attention (stage B) is SCALAR-bound -- the EXP activations process
exactly the score elements at half the PE clock, so exp slightly exceeds
the PE's scores+AV work. Fusing the stages hides each bottleneck under the
other: QKV chunk c+1's matmul groups are interleaved at kv-block
granularity into attention chunk c's emission, giving the in-order tensor
queue filler work whenever the exp chain lags.

Other schedule notes:
- First-chunk X^T/Wq/Wk loads split per k-tile across three DMA queues
  (first matmul ~12us after launch, ~8us of that is the Tile preamble).
- Both heads of a pair share one [128, 2, 512] scores PSUM and one EXP.
- Softmax normalization pipelined one chunk behind: split exact DVE
  reciprocal (two 512-col halves, emitted mid-chunk), staging copies on
  vector+scalar, one K=64 selector-matmul broadcast + one ct multiply per
  head pair, consumers spread one-per-pair through the next chunk.
- All QKV psum drains on the vector engine: the scalar engine is reserved
  for EXP during attention.
- Output is bf16 and its DMA is spread over all three queues so the ~4MB
  store drains inside the projection instead of as a tail.
"""

import json
import os
import sys

for _p in ("/opt/trn_rl_repo",):
    if _p not in sys.path:
        sys.path.insert(0, _p)

import numpy as np

# ---------------------------------------------------------------- constants
B = 4
S = 2048
E = 1024
H = 16
DH = 64
HL = 8  # heads per core
DL = HL * DH  # 512, local head dim
P = 128
NCORES = 8
SCALE = 1.0 / 8.0  # 1/sqrt(DH)
NEG = -1.0e30

KT_E = E // P  # 8  k-tiles over embed dim
MT = DL // P  # 4  m-tiles over local head dim (2 heads per m-tile)
SC = S // 512  # 4  512-wide chunks over sequence
SB = S // P  # 16 128-blocks over sequence
KT_D = DL // P  # 4  k-tiles over local head dim (proj contraction)
MT_E = E // P  # 8  m-tiles over embed dim (proj output)
VW = DH + 1  # 65: V columns per head + ones column
NGRP = 12  # stage-A matmul groups per chunk (4m x {q,k} + 4 V blocks)


# ------------------------------------------------- BIR multi-wait splitting
# The walrus build here accepts one sync-wait command per instruction; Tile
# attaches every outstanding dependency to the consumer. Split extras into
# single-wait EventSemaphore instructions just before the consumer (same
# engine => same blocking behavior).
_syncfix_done = [False]


def _install_syncfix():
    if _syncfix_done[0]:
        return
    _syncfix_done[0] = True
    import concourse.bass_utils as bu

    counter = [0]

    def split_multiwait(bir_json):
        d = json.loads(bir_json)
        changed = False
        for fn in d.get("functions", []):
            for bb in fn.get("blocks", []):
                new_insts = []
                for inst in bb.get("instructions", []):
                    si = inst.get("sync_info")
                    waits = (si or {}).get("on_wait") or []
                    if len(waits) > 1:
                        changed = True
                        for w in waits[:-1]:
                            counter[0] += 1
                            new_insts.append(
                                {
                                    "debug": inst.get("debug"),
                                    "engine": inst["engine"],
                                    "ins": [],
                                    "name": f"WSPLIT-{counter[0]}",
                                    "opcode": "EventSemaphore",
                                    "outs": [],
                                    "sync_info": {"on_update": [], "on_wait": [w]},
                                }
                            )
                        si["on_wait"] = [waits[-1]]
                    new_insts.append(inst)
                bb["instructions"] = new_insts
        if not changed:
            return bir_json if isinstance(bir_json, bytes) else bir_json.encode()
        return json.dumps(d).encode()

    orig = bu.compile_bir_kernel

    def patched(bir_json, tmpdir, neff_name="file.neff"):
        return orig(split_multiwait(bir_json), tmpdir, neff_name)

    bu.compile_bir_kernel = patched
    try:
        import concourse.bass2jax as b2j

        if hasattr(b2j, "compile_bir_kernel"):
            b2j.compile_bir_kernel = patched
    except ImportError:
        pass


# ------------------------------------------------------------ kernel build
def build_nc():
    import concourse.bass as bass
    import concourse.tile as tile
    from concourse import mybir

    f32 = mybir.dt.float32
    bf16 = mybir.dt.bfloat16
    EXP = mybir.ActivationFunctionType.Exp
    IDENT = mybir.ActivationFunctionType.Identity

    nc = bass.Bass()

    xt_ext = nc.dram_tensor("xt", [E, S], bf16, kind="ExternalInput")
    wq_ext = nc.dram_tensor("wq", [E, DL], bf16, kind="ExternalInput")
    wk_ext = nc.dram_tensor("wk", [E, DL], bf16, kind="ExternalInput")
    wv_ext = nc.dram_tensor("wv", [E, DL], bf16, kind="ExternalInput")
    wo_ext = nc.dram_tensor("wo", [DL, E], bf16, kind="ExternalInput")
    mask_ext = nc.dram_tensor("mask", [P, P], f32, kind="ExternalInput")
    vones_ext = nc.dram_tensor("vones", [P, SB * HL], bf16, kind="ExternalInput")
    sel64_ext = nc.dram_tensor("sel64", [DH, P], bf16, kind="ExternalInput")
    out_ext = nc.dram_tensor("outp", [E, S], bf16, kind="ExternalOutput")

    xt_r = xt_ext.rearrange("(kt p) s -> p kt s", p=P)
    wq_r = wq_ext.rearrange("(kt p) d -> p kt d", p=P)
    wk_r = wk_ext.rearrange("(kt p) d -> p kt d", p=P)
    wv_r = wv_ext.rearrange("(kt p) d -> p kt d", p=P)
    wo_r = wo_ext.rearrange("(kt p) e -> p kt e", p=P)

    with tile.TileContext(nc) as tc:
        with tc.tile_pool(name="persist", bufs=1) as pers:
            # ---- persistent SBUF tensors
            qt = [pers.tile([P, S], bf16, tag=f"qt{m}", name=f"qt{m}") for m in range(MT)]
            kt = [pers.tile([P, S], bf16, tag=f"kt{m}", name=f"kt{m}") for m in range(MT)]
            v_sb = pers.tile([P, SB, HL * VW], bf16, tag="v")
            ct = [
                pers.tile([P, KT_D, 512], bf16, tag=f"ct{i}", name=f"ct{i}")
                for i in range(SC)
            ]
            wo_sb = pers.tile([P, KT_D, E], bf16, tag="wo")
            mask_sb = pers.tile([P, P], f32, tag="mask")
            mask2_sb = pers.tile([P, 2, P], f32, tag="mask2")
            sel64_sb = pers.tile([DH, P], bf16, tag="sel64")
            # reciprocal staging rows for the selector matmul: rows 0 and 32
            # (legal engine partition bases) of 8 ring slots; other rows stay
            # zero so the K=64 contraction only picks up rows 0/32
            recr64 = pers.tile([DH, 8, 512], bf16, tag="recr64")
            scr_i = pers.tile([1, DH], f32, tag="scr_i")
            scr_o = pers.tile([1, DH], f32, tag="scr_o")

            with (
                tc.tile_pool(name="wqkv", bufs=1) as wpool,
                tc.tile_pool(name="xt", bufs=4) as xpool,
                tc.tile_pool(name="probs", bufs=6) as ppool,
                tc.tile_pool(name="fin", bufs=4) as fpool,
                tc.tile_pool(name="ostage", bufs=4) as opool,
                tc.tile_pool(name="ps_a", bufs=2, space="PSUM") as ps_a,
                tc.tile_pool(name="ps_s", bufs=2, space="PSUM") as ps_s,
                tc.tile_pool(name="ps_ctx", bufs=2, space="PSUM") as ps_ctx,
            ):
                wq_sb = wpool.tile([P, KT_E, DL], bf16, tag="wq")
                wk_sb = wpool.tile([P, KT_E, DL], bf16, tag="wk")
                wv_sb = wpool.tile([P, KT_E, DL], bf16, tag="wv")
                xc = [
                    xpool.tile([P, KT_E, 512], bf16, tag="xt", name=f"xt{c}")
                    for c in range(SC)
                ]
                # First-chunk operands split in k-slices across three queues
                # so the first matmul starts after ~256KB instead of ~3MB.
                for k2, ke in ((0, 1), (1, 3), (3, 5), (5, 8)):
                    nc.sync.dma_start(xc[0][:, k2:ke, :], xt_r[:, k2:ke, 0:512])
                    nc.gpsimd.dma_start(wq_sb[:, k2:ke, :], wq_r[:, k2:ke, :])
                    nc.scalar.dma_start(wk_sb[:, k2:ke, :], wk_r[:, k2:ke, :])
                for c in range(1, SC):
                    nc.sync.dma_start(
                        xc[c][:], xt_r[:, :, 512 * c : 512 * (c + 1)]
                    )
                nc.gpsimd.dma_start(wv_sb[:], wv_r)
                nc.gpsimd.dma_start(mask_sb[:], mask_ext[:, :])
                ones_col = v_sb[:].rearrange("p sb (h c) -> p sb h c", c=VW)[
                    :, :, :, DH : DH + 1
                ]
                nc.sync.dma_start(
                    ones_col,
                    vones_ext.rearrange("p (sb h one) -> p sb h one", h=HL, one=1),
                )
                nc.gpsimd.dma_start(sel64_sb[:], sel64_ext[:, :])
                nc.sync.dma_start(wo_sb[:], wo_r)

                # zero the selector rhs ring; preload the EXP table during
                # stage A (the lazy load otherwise hits the first scores)
                nc.gpsimd.memset(recr64[:], 0.0)
                nc.gpsimd.memset(scr_i[:], 0.0)
                nc.scalar.activation(scr_o[:], scr_i[:], EXP)
                for i in range(2):
                    nc.vector.tensor_copy(mask2_sb[:, i, :], mask_sb[:])

                # ---------------- stage A chunk = 12 matmul groups --------
                def a_group(c, gi):
                    if gi < 8:
                        m, w = divmod(gi, 2)
                        w_sb, dst = (wq_sb, qt) if w == 0 else (wk_sb, kt)
                        ps = ps_a.tile(
                            [P, 512], f32, tag="ps_a", name=f"psa{c}_{gi}"
                        )
                        for k in range(KT_E):
                            nc.tensor.matmul(
                                ps[:],
                                w_sb[:, k, P * m : P * (m + 1)],
                                xc[c][:, k, :],
                                start=(k == 0),
                                stop=(k == KT_E - 1),
                            )
                        nc.vector.tensor_copy(
                            dst[m][:, 512 * c : 512 * (c + 1)], ps[:]
                        )
                    else:
                        s = gi - 8
                        sb = 4 * c + s
                        ps = ps_a.tile([P, 512], f32, tag="ps_a", name=f"psv{sb}")
                        for k in range(KT_E):
                            nc.tensor.matmul(
                                ps[:],
                                xc[c][:, k, P * s : P * (s + 1)],
                                wv_sb[:, k, :],
                                start=(k == 0),
                                stop=(k == KT_E - 1),
                            )
                        nc.scalar.copy(
                            v_sb[:, sb, :].rearrange("p (h w) -> p h w", w=VW)[
                                :, :, 0:DH
                            ],
                            ps[:].rearrange("p (h w) -> p h w", w=DH),
                        )

                # ---------------- projection group (stage C work) ---------
                qdma = [nc.sync, nc.scalar, nc.gpsimd]
                pj_n = [0]

                def p_group(m, s):
                    ps = ps_a.tile([P, 512], f32, tag="ps_a", name=f"psp{m}_{s}")
                    for k in range(KT_D):
                        nc.tensor.matmul(
                            ps[:],
                            wo_sb[:, k, P * m : P * (m + 1)],
                            ct[s][:, k, :],
                            start=(k == 0),
                            stop=(k == KT_D - 1),
                        )
                    ot = opool.tile([P, 512], bf16, tag="ostage", name=f"ot{m}_{s}")
                    nc.vector.tensor_copy(ot[:], ps[:])
                    qdma[pj_n[0] % 3].dma_start(
                        out_ext[P * m : P * (m + 1), 512 * s : 512 * (s + 1)],
                        ot[:],
                    )
                    pj_n[0] += 1

                # filler: a queue of deferred tensor-heavy thunks dripped in
                # per attention block so the in-order PE queue always has
                # work while the exp chain lags. `skip` delays the first
                # fill (e.g. until that chunk's X tile has surely landed).
                fill = {"q": [], "acc": 0.0, "rate": 0.0, "skip": 0}

                def set_filler(thunks, total_blocks, skip=0):
                    fill["q"] = list(thunks)
                    fill["acc"] = 0.0
                    fill["skip"] = skip
                    fill["rate"] = len(fill["q"]) / max(1, total_blocks - skip) * 1.02

                def filler():
                    if not fill["q"]:
                        return
                    if fill["skip"] > 0:
                        fill["skip"] -= 1
                        return
                    fill["acc"] += fill["rate"]
                    while fill["acc"] >= 1.0 and fill["q"]:
                        fill["q"].pop(0)()
                        fill["acc"] -= 1.0

                def flush_filler():
                    while fill["q"]:
                        fill["q"].pop(0)()

                # ---------------- softmax normalization helpers -----------
                def emit_recr(c, recs, pairs):
                    for t in pairs:
                        slot = (c % 2) * MT + t
                        for i, base in ((0, 0), (1, 32)):
                            h = 2 * t + i
                            src_ap = recs[
                                32 * (h % 4) : 32 * (h % 4) + 1,
                                512 * (h // 4) : 512 * (h // 4) + 512,
                            ]
                            dst_ap = recr64[base : base + 1, slot, :]
                            if i == 0:
                                nc.vector.tensor_copy(dst_ap, src_ap)
                            else:
                                nc.scalar.copy(dst_ap, src_ap)

                def emit_bc(c, pairs):
                    for t in pairs:
                        slot = (c % 2) * MT + t
                        bcp = ps_a.tile(
                            [P, 512], f32, tag="ps_a", name=f"bc{c}_{t}"
                        )
                        nc.tensor.matmul(
                            bcp[:],
                            sel64_sb[:, :],
                            recr64[:, slot, :],
                            start=True,
                            stop=True,
                        )
                        nc.vector.tensor_mul(
                            ct[c][:, t, :], ct[c][:, t, :], bcp[:]
                        )

                # ---------------- attention head pair ---------------------
                def b_pair(c, t, sums_sb):
                    heads = (2 * t, 2 * t + 1)
                    q_lo, q_hi = 512 * c, 512 * (c + 1)
                    nblk = 4 * c + 4
                    ctx_ps = {}
                    probs = {}
                    for h in heads:
                        ctx_ps[h] = ps_ctx.tile(
                            [VW, 512], f32, tag="ps_ctx", name=f"ctx_{c}_{h}"
                        )

                    def av_mm(h, j, first, last):
                        r = j - 4 * c
                        lo = P * r if r > 0 else 0
                        nc.tensor.matmul(
                            ctx_ps[h][:, lo:512],
                            v_sb[:, j, VW * h : VW * (h + 1)],
                            probs[j][:, h % 2, lo:512],
                            start=first,
                            stop=last,
                        )

                    for j in range(nblk):
                        r = j - 4 * c
                        slo = P * r if r > 0 else 0
                        ps = ps_s.tile(
                            [P, 2, 512], f32, tag="ps_s", name=f"pss_{c}_{t}_{j}"
                        )
                        for i, h in enumerate(heads):
                            hp = DH * i
                            nc.tensor.matmul(
                                ps[:, i, slo:512],
                                kt[t][hp : hp + DH, P * j : P * (j + 1)],
                                qt[t][hp : hp + DH, q_lo + slo : q_hi],
                                start=True,
                                stop=True,
                            )
                        if r >= 0:
                            nc.vector.tensor_add(
                                ps[:, :, P * r : P * (r + 1)],
                                ps[:, :, P * r : P * (r + 1)],
                                mask2_sb[:],
                            )
                        pr = ppool.tile(
                            [P, 2, 512], bf16, tag="probs", name=f"pr_{c}_{t}_{j}"
                        )
                        probs[j] = pr
                        nc.scalar.activation(
                            pr[:, :, slo:512], ps[:, :, slo:512], EXP, scale=SCALE
                        )
                        if j >= 1:
                            for h in heads:
                                av_mm(h, j - 1, first=(j == 1), last=False)
                        filler()
                    for h in heads:
                        av_mm(h, nblk - 1, first=(nblk == 1), last=True)

                    # stash unnormalized ctx + row sums (both vector; sums
                    # land at 32-aligned partitions x 2 col slots so one
                    # batched reciprocal serves all 8 heads)
                    for i, h in enumerate(heads):
                        hp = DH * i
                        nc.vector.tensor_copy(
                            ct[c][hp : hp + DH, t, :], ctx_ps[h][0:DH, :]
                        )
                        nc.vector.tensor_copy(
                            sums_sb[
                                32 * (h % 4) : 32 * (h % 4) + 1,
                                512 * (h // 4) : 512 * (h // 4) + 512,
                            ],
                            ctx_ps[h][DH : DH + 1, :],
                        )

                # ---------------- fused emission --------------------------
                # filler allocation (balancing each segment's tensor-engine
                # work against its scalar exp load): B0 gets A1 + first half
                # of A2, B1 the rest of A2 + half of A3, B2 the rest of A3,
                # B3 gets the s<=2 projection groups (their ct chunks are
                # normalized by then)
                # chunk 0 startup: its own groups interleave with its own
                # pairs (m0 q/k + all V first, then m_t q/k between pairs)
                # so the scalar engine starts chunk-0 exps inside the
                # DMA-limited window
                for gi in (0, 1, 8, 9, 10, 11):
                    a_group(0, gi)
                a_thunks = {
                    c: [
                        (lambda c_=c, gi_=gi: a_group(c_, gi_))
                        for gi in range(NGRP)
                    ]
                    for c in (1, 2, 3)
                }
                # NOTE: projection filler may only cover ct chunks whose
                # normalization multiplies are EMITTED before it (emission
                # order defines the dependency graph): s=0,1 are normalized
                # during attention chunks 1/2; s=2's bc goes out at t==1 of
                # chunk 3, so its groups are lumped at t==2/t==3.
                p_thunks = [
                    (lambda m_=m, s_=s: p_group(m_, s_))
                    for s in range(2)
                    for m in range(MT_E)
                ]
                seg_fill = {
                    0: a_thunks[1] + a_thunks[2][:6],
                    1: a_thunks[2][6:] + a_thunks[3][:6],
                    2: a_thunks[3][6:],
                    3: p_thunks,
                }

                pending = None
                for c in range(SC):
                    sums_sb = fpool.tile(
                        [P, 1024], f32, tag="sums", name=f"sums{c}", bufs=2
                    )
                    recs = fpool.tile(
                        [P, 1024], f32, tag="recs", name=f"recs{c}", bufs=2
                    )
                    set_filler(seg_fill[c], 4 * (4 * c + 4), skip=8 if c == 0 else 0)
                    for t in range(MT):
                        b_pair(c, t, sums_sb)
                        if c == 0 and t < MT - 1:
                            a_group(0, 2 * t + 2)
                            a_group(0, 2 * t + 3)
                        if t == 1:
                            nc.vector.reciprocal(
                                recs[:, 0:512], sums_sb[:, 0:512]
                            )
                        if pending is not None:
                            if c == SC - 1:
                                # last chunk: consume all of chunk 2's norm
                                # at t==1 so the s=2 projection groups are
                                # legal from t==2 on
                                if t == 0:
                                    emit_recr(pending[0], pending[1], range(MT))
                                elif t == 1:
                                    emit_bc(pending[0], range(MT))
                                    pending = None
                            elif t == 0:
                                emit_recr(pending[0], pending[1], range(MT))
                            else:
                                emit_bc(pending[0], [t - 1])
                        if c == SC - 1 and t >= 2:
                            # s=2 projection in two lumps, then normalize
                            # this chunk's pairs 0/1 under pairs 2/3
                            for m in range(4 * (t - 2), 4 * (t - 1)):
                                p_group(m, 2)
                            emit_recr(c, recs, [t - 2])
                            emit_bc(c, [t - 2])
                    nc.vector.reciprocal(recs[:, 512:1024], sums_sb[:, 512:1024])
                    flush_filler()
                    if pending is not None:
                        emit_bc(pending[0], [MT - 1])
                    pending = (c, recs)

                # tail: normalize the last chunk's pairs 2/3, then the s=3
                # projection groups (k ascending hits pairs 2/3 last)
                c_f, recs_f = pending
                for t in (2, 3):
                    emit_recr(c_f, recs_f, [t])
                    emit_bc(c_f, [t])
                for m in range(MT_E):
                    p_group(m, SC - 1)

    return nc


# ------------------------------------------------------------ PJRT runner
class _Runner:
    """Compile once, run many: mirrors bass2jax.run_bass_via_pjrt with a
    cached jitted executable."""

    def __init__(self, nc):
        import jax
        import jax.numpy  # noqa: F401
        from jax.sharding import Mesh, PartitionSpec
        from jax.experimental.shard_map import shard_map
        import concourse.bass2jax as b2j
        from concourse import mybir

        b2j.install_neuronx_cc_hook()
        self.jax = jax
        partition_name = (
            nc.partition_id_tensor.name if nc.partition_id_tensor else None
        )
        in_names = []
        out_names = []
        out_avals = []
        self.zero_shapes = []
        for alloc in nc.m.functions[0].allocations:
            if not isinstance(alloc, mybir.MemoryLocationSet):
                continue
            name = alloc.memorylocations[0].name
            if alloc.kind == "ExternalInput":
                if name == partition_name:
                    continue
                in_names.append(name)
            elif alloc.kind == "ExternalOutput":
                shape = tuple(alloc.tensor_shape)
                dtype = mybir.dt.np(alloc.dtype)
                out_names.append(name)
                out_avals.append(jax.core.ShapedArray(shape, dtype))
                self.zero_shapes.append((shape, dtype))
        self.in_names = in_names
        self.out_names = out_names
        self.out_avals = out_avals
        n_params = len(in_names)
        n_outs = len(out_avals)
        all_in = list(in_names) + list(out_names)
        if partition_name is not None:
            all_in.append(partition_name)

        def _body(*args):
            operands = list(args)
            if partition_name is not None:
                operands.append(b2j.partition_id_tensor())
            outs = b2j._bass_exec_p.bind(
                *operands,
                out_avals=tuple(out_avals),
                in_names=tuple(all_in),
                out_names=tuple(out_names),
                lowering_input_output_aliases=(),
                sim_require_finite=True,
                sim_require_nnan=True,
                nc=nc,
            )
            return tuple(outs)

        devices = jax.devices()[:NCORES]
        assert len(devices) == NCORES, f"need {NCORES} cores, got {len(devices)}"
        self.mesh = Mesh(np.asarray(devices), ("core",))
        in_specs = (PartitionSpec("core"),) * (n_params + n_outs)
        out_specs = (PartitionSpec("core"),) * n_outs
        self.fn = jax.jit(
            shard_map(
                _body,
                mesh=self.mesh,
                in_specs=in_specs,
                out_specs=out_specs,
                check_rep=False,
            ),
            donate_argnums=tuple(range(n_params, n_params + n_outs)),
            keep_unused=True,
        )

    def run(self, in_maps):
        concat_in = [
            np.concatenate([np.asarray(m[name]) for m in in_maps], axis=0)
            for name in self.in_names
        ]
        zeros = [
            np.zeros((NCORES * s[0], *s[1:]), dt) for s, dt in self.zero_shapes
        ]
        outs = self.fn(*concat_in, *zeros)
        return [
            {
                name: np.asarray(outs[i]).reshape(
                    NCORES, *self.out_avals[i].shape
                )[c]
                for i, name in enumerate(self.out_names)
            }
            for c in range(NCORES)
        ]


_cache = {}


def _get_runner():
    if "runner" not in _cache:
        _install_syncfix()
        _cache["runner"] = _Runner(build_nc())
    return _cache["runner"]


def make_in_maps(X, Wq, Wk, Wv, Wo, bo):
    import ml_dtypes

    bf16 = ml_dtypes.bfloat16
    X = np.asarray(X, dtype=np.float32)
    Wq = np.asarray(Wq, dtype=np.float32)
    Wk = np.asarray(Wk, dtype=np.float32)
    Wv = np.asarray(Wv, dtype=np.float32)
    Wo = np.asarray(Wo, dtype=np.float32)
    bo = np.asarray(bo, dtype=np.float32)

    kv = np.arange(P)[:, None]
    qq = np.arange(P)[None, :]
    mask = np.where(kv > qq, np.float32(NEG), np.float32(0.0))

    sel64 = np.zeros((DH, P), dtype=bf16)
    sel64[0, 0:DH] = 1
    sel64[32, DH:P] = 1

    in_maps = []
    for core in range(NCORES):
        b, g = divmod(core, 2)
        h0 = HL * g
        in_maps.append(
            {
                "xt": np.ascontiguousarray(X[b].T).astype(bf16),
                "wq": np.ascontiguousarray(
                    Wq[h0 : h0 + HL].transpose(1, 0, 2).reshape(E, DL)
                ).astype(bf16),
                "wk": np.ascontiguousarray(
                    Wk[h0 : h0 + HL].transpose(1, 0, 2).reshape(E, DL)
                ).astype(bf16),
                "wv": np.ascontiguousarray(
                    Wv[h0 : h0 + HL].transpose(1, 0, 2).reshape(E, DL)
                ).astype(bf16),
                "wo": np.ascontiguousarray(Wo[:, DL * g : DL * (g + 1)].T).astype(
                    bf16
                ),
                "mask": mask,
                "vones": np.ones((P, SB * HL), dtype=bf16),
                "sel64": sel64,
            }
        )
    return in_maps


def assemble(results, bo):
    bo = np.asarray(bo, dtype=np.float32)
    out = np.empty((B, S, E), dtype=np.float32)
    for b in range(B):
        acc = results[2 * b]["outp"].astype(np.float32) + results[
            2 * b + 1
        ]["outp"].astype(np.float32)
        out[b] = acc.T + bo
    return out


def kernel(X, Wq, Wk, Wv, Wo, bo):
    runner = _get_runner()
    in_maps = make_in_maps(X, Wq, Wk, Wv, Wo, bo)
    results = runner.run(in_maps)
    return assemble(results, bo)


# revision 25
# speedup vs baseline: 1.1319x; 1.1319x over previous
"""Multi-head causal attention (B=4, S=2048, E=1024, H=16, Dh=64) on 8
Trainium2 NeuronCores.

Sharding: data-parallel over the 4 batch elements x tensor-parallel over
heads (2 groups of 8). Core 2b+g handles batch b, heads 8g..8g+7. Each core
computes Q^T/K^T (head dim on partitions), V (natural layout, with a fused
ones-column so the attention-weight row sums fall out of the same matmul),
block-causal scores in transposed [kv, q] layout (no transposes needed:
softmax normalization is a reciprocal + selector-matmul partition
broadcast), the local-head context, and the output projection against its
slice of Wo. The two partial projections per batch are summed on the host
(the TP "all-reduce" of the sharding hint, done at gather time), which also
absorbs the out-transpose: the kernel emits out^T [E, S] in bf16.

All matmuls run in bf16 (fp32 accumulate): f32r streams at the same 1
cycle/row but draws enough power that the core throttles to a ~50%
utilization cap for half the runtime; bf16 also trims the r=3 diagonal
scores block to its live 128 columns (no N>=256 full-rate requirement).

KEY SCHEDULE IDEA (from per-engine totals of the unfused ancestors): the
QKV projections (stage A) are tensor-bound with an idle scalar engine,
while attention (stage B) is SCALAR-bound -- the EXP activations process
exactly the score elements at half the PE clock, so exp slightly exceeds
the PE's scores+AV work. Fusing the stages hides each bottleneck under the
other: QKV chunk c+1's matmul groups are interleaved at kv-block
granularity into attention chunk c's emission, giving the in-order tensor
queue filler work whenever the exp chain lags.

Other schedule notes:
- First-chunk X^T/Wq/Wk loads split per k-tile across three DMA queues
  (first matmul ~12us after launch, ~8us of that is the Tile preamble).
- Both heads of a pair share one [128, 2, 512] scores PSUM and one EXP.
- Softmax normalization pipelined one chunk behind: split exact DVE
  reciprocal (two 512-col halves, emitted mid-chunk), staging copies on
  vector+scalar, one K=64 selector-matmul broadcast + one ct multiply per
  head pair, consumers spread one-per-pair through the next chunk.
- All QKV psum drains on the vector engine: the scalar engine is reserved
  for EXP during attention.
- Output is bf16 and its DMA is spread over all three queues so the ~4MB
  store drains inside the projection instead of as a tail.
"""

import json
import os
import sys

for _p in ("/opt/trn_rl_repo",):
    if _p not in sys.path:
        sys.path.insert(0, _p)

import numpy as np

# ---------------------------------------------------------------- constants
B = 4
S = 2048
E = 1024
H = 16
DH = 64
HL = 8  # heads per core
DL = HL * DH  # 512, local head dim
P = 128
NCORES = 8
SCALE = 1.0 / 8.0  # 1/sqrt(DH)
NEG = -1.0e30

KT_E = E // P  # 8  k-tiles over embed dim
MT = DL // P  # 4  m-tiles over local head dim (2 heads per m-tile)
SC = S // 512  # 4  512-wide chunks over sequence
SB = S // P  # 16 128-blocks over sequence
KT_D = DL // P  # 4  k-tiles over local head dim (proj contraction)
MT_E = E // P  # 8  m-tiles over embed dim (proj output)
VW = DH + 1  # 65: V columns per head + ones column
NGRP = 12  # stage-A matmul groups per chunk (4m x {q,k} + 4 V blocks)


# ------------------------------------------------- BIR multi-wait splitting
# The walrus build here accepts one sync-wait command per instruction; Tile
# attaches every outstanding dependency to the consumer. Split extras into
# single-wait EventSemaphore instructions just before the consumer (same
# engine => same blocking behavior).
_syncfix_done = [False]


def _install_syncfix():
    if _syncfix_done[0]:
        return
    _syncfix_done[0] = True
    import concourse.bass_utils as bu

    counter = [0]

    def split_multiwait(bir_json):
        d = json.loads(bir_json)
        changed = False
        for fn in d.get("functions", []):
            for bb in fn.get("blocks", []):
                new_insts = []
                for inst in bb.get("instructions", []):
                    si = inst.get("sync_info")
                    waits = (si or {}).get("on_wait") or []
                    if len(waits) > 1:
                        changed = True
                        for w in waits[:-1]:
                            counter[0] += 1
                            new_insts.append(
                                {
                                    "debug": inst.get("debug"),
                                    "engine": inst["engine"],
                                    "ins": [],
                                    "name": f"WSPLIT-{counter[0]}",
                                    "opcode": "EventSemaphore",
                                    "outs": [],
                                    "sync_info": {"on_update": [], "on_wait": [w]},
                                }
                            )
                        si["on_wait"] = [waits[-1]]
                    new_insts.append(inst)
                bb["instructions"] = new_insts
        if not changed:
            return bir_json if isinstance(bir_json, bytes) else bir_json.encode()
        return json.dumps(d).encode()

    orig = bu.compile_bir_kernel

    def patched(bir_json, tmpdir, neff_name="file.neff"):
        return orig(split_multiwait(bir_json), tmpdir, neff_name)

    bu.compile_bir_kernel = patched
    try:
        import concourse.bass2jax as b2j

        if hasattr(b2j, "compile_bir_kernel"):
            b2j.compile_bir_kernel = patched
    except ImportError:
        pass


# ------------------------------------------------------------ kernel build
def build_nc():
    import concourse.bass as bass
    import concourse.tile as tile
    from concourse import mybir

    f32 = mybir.dt.float32
    bf16 = mybir.dt.bfloat16
    EXP = mybir.ActivationFunctionType.Exp
    IDENT = mybir.ActivationFunctionType.Identity

    nc = bass.Bass()

    xt_ext = nc.dram_tensor("xt", [E, S], bf16, kind="ExternalInput")
    wq_ext = nc.dram_tensor("wq", [E, DL], bf16, kind="ExternalInput")
    wk_ext = nc.dram_tensor("wk", [E, DL], bf16, kind="ExternalInput")
    wv_ext = nc.dram_tensor("wv", [E, DL], bf16, kind="ExternalInput")
    wo_ext = nc.dram_tensor("wo", [DL, E], bf16, kind="ExternalInput")
    bo_ext = nc.dram_tensor("bo2", [E], f32, kind="ExternalInput")
    mask_ext = nc.dram_tensor("mask", [P, P], f32, kind="ExternalInput")
    vones_ext = nc.dram_tensor("vones", [P, SB * HL], bf16, kind="ExternalInput")
    sel64_ext = nc.dram_tensor("sel64", [DH, P], bf16, kind="ExternalInput")
    out_ext = nc.dram_tensor("outp", [E, S], bf16, kind="ExternalOutput")

    xt_r = xt_ext.rearrange("(kt p) s -> p kt s", p=P)
    wq_r = wq_ext.rearrange("(kt p) d -> p kt d", p=P)
    wk_r = wk_ext.rearrange("(kt p) d -> p kt d", p=P)
    wv_r = wv_ext.rearrange("(kt p) d -> p kt d", p=P)
    wo_r = wo_ext.rearrange("(kt p) e -> p kt e", p=P)
    bo_r = bo_ext.rearrange("(m p) -> p m", p=P)

    with tile.TileContext(nc) as tc:
        with tc.tile_pool(name="persist", bufs=1) as pers:
            # ---- persistent SBUF tensors
            qt = [pers.tile([P, S], bf16, tag=f"qt{m}", name=f"qt{m}") for m in range(MT)]
            kt = [pers.tile([P, S], bf16, tag=f"kt{m}", name=f"kt{m}") for m in range(MT)]
            v_sb = pers.tile([P, SB, HL * VW], bf16, tag="v")
            ct = [
                pers.tile([P, KT_D, 512], bf16, tag=f"ct{i}", name=f"ct{i}")
                for i in range(SC)
            ]
            wo_sb = pers.tile([P, KT_D, E], bf16, tag="wo")
            bo_sb = pers.tile([P, MT_E], f32, tag="bo")
            mask_sb = pers.tile([P, P], f32, tag="mask")
            mask2_sb = pers.tile([P, 2, P], f32, tag="mask2")
            sel64_sb = pers.tile([DH, P], bf16, tag="sel64")
            # reciprocal staging rows for the selector matmul: rows 0 and 32
            # (legal engine partition bases) of 8 ring slots; other rows stay
            # zero so the K=64 contraction only picks up rows 0/32
            recr64 = pers.tile([DH, 8, 512], bf16, tag="recr64")
            scr_i = pers.tile([1, DH], f32, tag="scr_i")
            scr_o = pers.tile([1, DH], f32, tag="scr_o")

            with (
                tc.tile_pool(name="wqkv", bufs=1) as wpool,
                tc.tile_pool(name="xt", bufs=4) as xpool,
                tc.tile_pool(name="probs", bufs=6) as ppool,
                tc.tile_pool(name="fin", bufs=4) as fpool,
                tc.tile_pool(name="ps_a", bufs=2, space="PSUM") as ps_a,
                tc.tile_pool(name="ps_s", bufs=2, space="PSUM") as ps_s,
                tc.tile_pool(name="ps_ctx", bufs=2, space="PSUM") as ps_ctx,
            ):
                wq_sb = wpool.tile([P, KT_E, DL], bf16, tag="wq")
                wk_sb = wpool.tile([P, KT_E, DL], bf16, tag="wk")
                wv_sb = wpool.tile([P, KT_E, DL], bf16, tag="wv")
                xc = [
                    xpool.tile([P, KT_E, 512], bf16, tag="xt", name=f"xt{c}")
                    for c in range(SC)
                ]
                # First-chunk operands split in k-slices across three queues
                # so the first matmul starts after ~256KB instead of ~3MB.
                for k2, ke in ((0, 1), (1, 3), (3, 5), (5, 8)):
                    nc.sync.dma_start(xc[0][:, k2:ke, :], xt_r[:, k2:ke, 0:512])
                    nc.gpsimd.dma_start(wq_sb[:, k2:ke, :], wq_r[:, k2:ke, :])
                    nc.scalar.dma_start(wk_sb[:, k2:ke, :], wk_r[:, k2:ke, :])
                for c in range(1, SC):
                    nc.sync.dma_start(
                        xc[c][:], xt_r[:, :, 512 * c : 512 * (c + 1)]
                    )
                nc.gpsimd.dma_start(wv_sb[:], wv_r)
                nc.gpsimd.dma_start(mask_sb[:], mask_ext[:, :])
                ones_col = v_sb[:].rearrange("p sb (h c) -> p sb h c", c=VW)[
                    :, :, :, DH : DH + 1
                ]
                nc.sync.dma_start(
                    ones_col,
                    vones_ext.rearrange("p (sb h one) -> p sb h one", h=HL, one=1),
                )
                nc.gpsimd.dma_start(sel64_sb[:], sel64_ext[:, :])
                nc.gpsimd.dma_start(bo_sb[:], bo_r)
                nc.sync.dma_start(wo_sb[:], wo_r)

                # zero the selector rhs ring; preload the EXP table during
                # stage A (the lazy load otherwise hits the first scores)
                nc.gpsimd.memset(recr64[:], 0.0)
                nc.gpsimd.memset(scr_i[:], 0.0)
                nc.scalar.activation(scr_o[:], scr_i[:], EXP)
                for i in range(2):
                    nc.vector.tensor_copy(mask2_sb[:, i, :], mask_sb[:])

                # ---------------- stage A chunk = 12 matmul groups --------
                def a_group(c, gi):
                    if gi < 8:
                        m, w = divmod(gi, 2)
                        w_sb, dst = (wq_sb, qt) if w == 0 else (wk_sb, kt)
                        ps = ps_a.tile(
                            [P, 512], f32, tag="ps_a", name=f"psa{c}_{gi}"
                        )
                        for k in range(KT_E):
                            nc.tensor.matmul(
                                ps[:],
                                w_sb[:, k, P * m : P * (m + 1)],
                                xc[c][:, k, :],
                                start=(k == 0),
                                stop=(k == KT_E - 1),
                            )
                        nc.vector.tensor_copy(
                            dst[m][:, 512 * c : 512 * (c + 1)], ps[:]
                        )
                    else:
                        s = gi - 8
                        sb = 4 * c + s
                        ps = ps_a.tile([P, 512], f32, tag="ps_a", name=f"psv{sb}")
                        for k in range(KT_E):
                            nc.tensor.matmul(
                                ps[:],
                                xc[c][:, k, P * s : P * (s + 1)],
                                wv_sb[:, k, :],
                                start=(k == 0),
                                stop=(k == KT_E - 1),
                            )
                        nc.vector.tensor_copy(
                            v_sb[:, sb, :].rearrange("p (h w) -> p h w", w=VW)[
                                :, :, 0:DH
                            ],
                            ps[:].rearrange("p (h w) -> p h w", w=DH),
                        )

                # filler state: next chunk's A-groups dripped in per block
                fill = {"c": None, "gi": 0, "acc": 0.0, "rate": 0.0}

                def set_filler(c_next, total_blocks):
                    fill["c"] = c_next
                    fill["gi"] = 0
                    fill["acc"] = 0.0
                    fill["rate"] = (
                        NGRP / total_blocks * 1.02 if c_next is not None else 0.0
                    )

                def filler():
                    if fill["c"] is None or fill["gi"] >= NGRP:
                        return
                    fill["acc"] += fill["rate"]
                    while fill["acc"] >= 1.0 and fill["gi"] < NGRP:
                        a_group(fill["c"], fill["gi"])
                        fill["gi"] += 1
                        fill["acc"] -= 1.0

                def flush_filler():
                    while fill["c"] is not None and fill["gi"] < NGRP:
                        a_group(fill["c"], fill["gi"])
                        fill["gi"] += 1

                # ---------------- softmax normalization helpers -----------
                def emit_recr(c, recs, pairs):
                    for t in pairs:
                        slot = (c % 2) * MT + t
                        for i, base in ((0, 0), (1, 32)):
                            h = 2 * t + i
                            src_ap = recs[
                                32 * (h % 4) : 32 * (h % 4) + 1,
                                512 * (h // 4) : 512 * (h // 4) + 512,
                            ]
                            dst_ap = recr64[base : base + 1, slot, :]
                            if i == 0:
                                nc.vector.tensor_copy(dst_ap, src_ap)
                            else:
                                nc.scalar.copy(dst_ap, src_ap)

                def emit_bc(c, pairs):
                    for t in pairs:
                        slot = (c % 2) * MT + t
                        bcp = ps_a.tile(
                            [P, 512], f32, tag="ps_a", name=f"bc{c}_{t}"
                        )
                        nc.tensor.matmul(
                            bcp[:],
                            sel64_sb[:, :],
                            recr64[:, slot, :],
                            start=True,
                            stop=True,
                        )
                        nc.vector.tensor_mul(
                            ct[c][:, t, :], ct[c][:, t, :], bcp[:]
                        )

                # ---------------- attention head pair ---------------------
                def b_pair(c, t, sums_sb):
                    heads = (2 * t, 2 * t + 1)
                    q_lo, q_hi = 512 * c, 512 * (c + 1)
                    nblk = 4 * c + 4
                    ctx_ps = {}
                    probs = {}
                    for h in heads:
                        ctx_ps[h] = ps_ctx.tile(
                            [VW, 512], f32, tag="ps_ctx", name=f"ctx_{c}_{h}"
                        )

                    def av_mm(h, j, first, last):
                        r = j - 4 * c
                        lo = P * r if r > 0 else 0
                        nc.tensor.matmul(
                            ctx_ps[h][:, lo:512],
                            v_sb[:, j, VW * h : VW * (h + 1)],
                            probs[j][:, h % 2, lo:512],
                            start=first,
                            stop=last,
                        )

                    for j in range(nblk):
                        r = j - 4 * c
                        slo = P * r if r > 0 else 0
                        ps = ps_s.tile(
                            [P, 2, 512], f32, tag="ps_s", name=f"pss_{c}_{t}_{j}"
                        )
                        for i, h in enumerate(heads):
                            hp = DH * i
                            nc.tensor.matmul(
                                ps[:, i, slo:512],
                                kt[t][hp : hp + DH, P * j : P * (j + 1)],
                                qt[t][hp : hp + DH, q_lo + slo : q_hi],
                                start=True,
                                stop=True,
                            )
                        if r >= 0:
                            nc.vector.tensor_add(
                                ps[:, :, P * r : P * (r + 1)],
                                ps[:, :, P * r : P * (r + 1)],
                                mask2_sb[:],
                            )
                        pr = ppool.tile(
                            [P, 2, 512], bf16, tag="probs", name=f"pr_{c}_{t}_{j}"
                        )
                        probs[j] = pr
                        nc.scalar.activation(
                            pr[:, :, slo:512], ps[:, :, slo:512], EXP, scale=SCALE
                        )
                        if j >= 1:
                            for h in heads:
                                av_mm(h, j - 1, first=(j == 1), last=False)
                        filler()
                    for h in heads:
                        av_mm(h, nblk - 1, first=(nblk == 1), last=True)

                    # stash unnormalized ctx + row sums (both vector; sums
                    # land at 32-aligned partitions x 2 col slots so one
                    # batched reciprocal serves all 8 heads)
                    for i, h in enumerate(heads):
                        hp = DH * i
                        nc.vector.tensor_copy(
                            ct[c][hp : hp + DH, t, :], ctx_ps[h][0:DH, :]
                        )
                        nc.vector.tensor_copy(
                            sums_sb[
                                32 * (h % 4) : 32 * (h % 4) + 1,
                                512 * (h // 4) : 512 * (h // 4) + 512,
                            ],
                            ctx_ps[h][DH : DH + 1, :],
                        )

                # ---------------- fused emission --------------------------
                for gi in range(NGRP):
                    a_group(0, gi)

                pending = None
                for c in range(SC):
                    sums_sb = fpool.tile(
                        [P, 1024], f32, tag="sums", name=f"sums{c}", bufs=2
                    )
                    recs = fpool.tile(
                        [P, 1024], f32, tag="recs", name=f"recs{c}", bufs=2
                    )
                    set_filler(c + 1 if c + 1 < SC else None, 4 * (4 * c + 4))
                    for t in range(MT):
                        b_pair(c, t, sums_sb)
                        if t == 1:
                            nc.vector.reciprocal(
                                recs[:, 0:512], sums_sb[:, 0:512]
                            )
                        if pending is not None:
                            if t == 0:
                                emit_recr(pending[0], pending[1], range(MT))
                            else:
                                emit_bc(pending[0], [t - 1])
                    nc.vector.reciprocal(recs[:, 512:1024], sums_sb[:, 512:1024])
                    flush_filler()
                    if pending is not None:
                        emit_bc(pending[0], [MT - 1])
                    pending = (c, recs)

                # final chunk: per-pair order so pairs 0/1 (half-0
                # reciprocal, long done) broadcast while half 1 finishes
                c_f, recs_f = pending
                for t in range(MT):
                    emit_recr(c_f, recs_f, [t])
                    emit_bc(c_f, [t])

            # ---- stage C: out^T = WoT-slice.T @ ctx^T (+ bias, group 0)
            with (
                tc.tile_pool(name="ostage", bufs=4) as opool,
                tc.tile_pool(name="ps_p", bufs=4, space="PSUM") as ps_p,
            ):
                qdma = [nc.sync, nc.scalar, nc.gpsimd]
                for m in range(MT_E):
                    for s in range(SC):
                        ps = ps_p.tile([P, 512], f32, tag="ps_p", name=f"psp{m}_{s}")
                        for k in range(KT_D):
                            nc.tensor.matmul(
                                ps[:],
                                wo_sb[:, k, P * m : P * (m + 1)],
                                ct[s][:, k, :],
                                start=(k == 0),
                                stop=(k == KT_D - 1),
                            )
                        ot = opool.tile(
                            [P, 512], bf16, tag="ostage", name=f"ot{m}_{s}"
                        )
                        nc.scalar.activation(
                            ot[:], ps[:], IDENT, bias=bo_sb[:, m : m + 1]
                        )
                        qdma[(m * SC + s) % 3].dma_start(
                            out_ext[P * m : P * (m + 1), 512 * s : 512 * (s + 1)],
                            ot[:],
                        )

    return nc


# ------------------------------------------------------------ PJRT runner
class _Runner:
    """Compile once, run many: mirrors bass2jax.run_bass_via_pjrt with a
    cached jitted executable."""

    def __init__(self, nc):
        import jax
        import jax.numpy  # noqa: F401
        from jax.sharding import Mesh, PartitionSpec
        from jax.experimental.shard_map import shard_map
        import concourse.bass2jax as b2j
        from concourse import mybir

        b2j.install_neuronx_cc_hook()
        self.jax = jax
        partition_name = (
            nc.partition_id_tensor.name if nc.partition_id_tensor else None
        )
        in_names = []
        out_names = []
        out_avals = []
        self.zero_shapes = []
        for alloc in nc.m.functions[0].allocations:
            if not isinstance(alloc, mybir.MemoryLocationSet):
                continue
            name = alloc.memorylocations[0].name
            if alloc.kind == "ExternalInput":
                if name == partition_name:
                    continue
                in_names.append(name)
            elif alloc.kind == "ExternalOutput":
                shape = tuple(alloc.tensor_shape)
                dtype = mybir.dt.np(alloc.dtype)
                out_names.append(name)
                out_avals.append(jax.core.ShapedArray(shape, dtype))
                self.zero_shapes.append((shape, dtype))
        self.in_names = in_names
        self.out_names = out_names
        self.out_avals = out_avals
        n_params = len(in_names)
        n_outs = len(out_avals)
        all_in = list(in_names) + list(out_names)
        if partition_name is not None:
            all_in.append(partition_name)

        def _body(*args):
            operands = list(args)
            if partition_name is not None:
                operands.append(b2j.partition_id_tensor())
            outs = b2j._bass_exec_p.bind(
                *operands,
                out_avals=tuple(out_avals),
                in_names=tuple(all_in),
                out_names=tuple(out_names),
                lowering_input_output_aliases=(),
                sim_require_finite=True,
                sim_require_nnan=True,
                nc=nc,
            )
            return tuple(outs)

        devices = jax.devices()[:NCORES]
        assert len(devices) == NCORES, f"need {NCORES} cores, got {len(devices)}"
        self.mesh = Mesh(np.asarray(devices), ("core",))
        in_specs = (PartitionSpec("core"),) * (n_params + n_outs)
        out_specs = (PartitionSpec("core"),) * n_outs
        self.fn = jax.jit(
            shard_map(
                _body,
                mesh=self.mesh,
                in_specs=in_specs,
                out_specs=out_specs,
                check_rep=False,
            ),
            donate_argnums=tuple(range(n_params, n_params + n_outs)),
            keep_unused=True,
        )

    def run(self, in_maps):
        concat_in = [
            np.concatenate([np.asarray(m[name]) for m in in_maps], axis=0)
            for name in self.in_names
        ]
        zeros = [
            np.zeros((NCORES * s[0], *s[1:]), dt) for s, dt in self.zero_shapes
        ]
        outs = self.fn(*concat_in, *zeros)
        return [
            {
                name: np.asarray(outs[i]).reshape(
                    NCORES, *self.out_avals[i].shape
                )[c]
                for i, name in enumerate(self.out_names)
            }
            for c in range(NCORES)
        ]


_cache = {}


def _get_runner():
    if "runner" not in _cache:
        _install_syncfix()
        _cache["runner"] = _Runner(build_nc())
    return _cache["runner"]


def make_in_maps(X, Wq, Wk, Wv, Wo, bo):
    import ml_dtypes

    bf16 = ml_dtypes.bfloat16
    X = np.asarray(X, dtype=np.float32)
    Wq = np.asarray(Wq, dtype=np.float32)
    Wk = np.asarray(Wk, dtype=np.float32)
    Wv = np.asarray(Wv, dtype=np.float32)
    Wo = np.asarray(Wo, dtype=np.float32)
    bo = np.asarray(bo, dtype=np.float32)

    kv = np.arange(P)[:, None]
    qq = np.arange(P)[None, :]
    mask = np.where(kv > qq, np.float32(NEG), np.float32(0.0))

    sel64 = np.zeros((DH, P), dtype=bf16)
    sel64[0, 0:DH] = 1
    sel64[32, DH:P] = 1

    in_maps = []
    for core in range(NCORES):
        b, g = divmod(core, 2)
        h0 = HL * g
        in_maps.append(
            {
                "xt": np.ascontiguousarray(X[b].T).astype(bf16),
                "wq": np.ascontiguousarray(
                    Wq[h0 : h0 + HL].transpose(1, 0, 2).reshape(E, DL)
                ).astype(bf16),
                "wk": np.ascontiguousarray(
                    Wk[h0 : h0 + HL].transpose(1, 0, 2).reshape(E, DL)
                ).astype(bf16),
                "wv": np.ascontiguousarray(
                    Wv[h0 : h0 + HL].transpose(1, 0, 2).reshape(E, DL)
                ).astype(bf16),
                "wo": np.ascontiguousarray(Wo[:, DL * g : DL * (g + 1)].T).astype(
                    bf16
                ),
                "bo2": bo if g == 0 else np.zeros_like(bo),
                "mask": mask,
                "vones": np.ones((P, SB * HL), dtype=bf16),
                "sel64": sel64,
            }
        )
    return in_maps


def assemble(results):
    out = np.empty((B, S, E), dtype=np.float32)
    for b in range(B):
        acc = results[2 * b]["outp"].astype(np.float32) + results[
            2 * b + 1
        ]["outp"].astype(np.float32)
        out[b] = acc.T
    return out


def kernel(X, Wq, Wk, Wv, Wo, bo):
    runner = _get_runner()
    in_maps = make_in_maps(X, Wq, Wk, Wv, Wo, bo)
    results = runner.run(in_maps)
    return assemble(results)


# revision 26
# speedup vs baseline: 1.1497x; 1.0158x over previous
"""Multi-head causal attention (B=4, S=2048, E=1024, H=16, Dh=64) on 8
Trainium2 NeuronCores.

Sharding: data-parallel over the 4 batch elements x tensor-parallel over
heads (2 groups of 8). Core 2b+g handles batch b, heads 8g..8g+7. Each core
computes Q^T/K^T (head dim on partitions), V (natural layout, with a fused
ones-column so the attention-weight row sums fall out of the same matmul),
block-causal scores in transposed [kv, q] layout (no transposes needed:
softmax normalization is a reciprocal + selector-matmul partition
broadcast), the local-head context, and the output projection against its
slice of Wo. The two partial projections per batch are summed on the host
(the TP "all-reduce" of the sharding hint, done at gather time), which also
absorbs the out-transpose: the kernel emits out^T [E, S] in bf16.

All matmuls run in bf16 (fp32 accumulate): f32r streams at the same 1
cycle/row but draws enough power that the core throttles to a ~50%
utilization cap for half the runtime; bf16 also trims the r=3 diagonal
scores block to its live 128 columns (no N>=256 full-rate requirement).

KEY SCHEDULE IDEA (from per-engine totals of the unfused ancestors): the
QKV projections (stage A) are tensor-bound with an idle scalar engine,
while attention (stage B) is SCALAR-bound -- the EXP activations process
exactly the score elements at half the PE clock, so exp slightly exceeds
the PE's scores+AV work. Fusing the stages hides each bottleneck under the
other: QKV chunk c+1's matmul groups are interleaved at kv-block
granularity into attention chunk c's emission, giving the in-order tensor
queue filler work whenever the exp chain lags.

Other schedule notes:
- First-chunk X^T/Wq/Wk loads split per k-tile across three DMA queues
  (first matmul ~12us after launch, ~8us of that is the Tile preamble).
- Both heads of a pair share one [128, 2, 512] scores PSUM and one EXP.
- Softmax normalization pipelined one chunk behind: split exact DVE
  reciprocal (two 512-col halves, emitted mid-chunk), staging copies on
  vector+scalar, one K=64 selector-matmul broadcast + one ct multiply per
  head pair, consumers spread one-per-pair through the next chunk.
- All QKV psum drains on the vector engine: the scalar engine is reserved
  for EXP during attention.
- Output is bf16 and its DMA is spread over all three queues so the ~4MB
  store drains inside the projection instead of as a tail.
"""

import json
import os
import sys

for _p in ("/opt/trn_rl_repo",):
    if _p not in sys.path:
        sys.path.insert(0, _p)

import numpy as np

# ---------------------------------------------------------------- constants
B = 4
S = 2048
E = 1024
H = 16
DH = 64
HL = 8  # heads per core
DL = HL * DH  # 512, local head dim
P = 128
NCORES = 8
SCALE = 1.0 / 8.0  # 1/sqrt(DH)
NEG = -1.0e30

KT_E = E // P  # 8  k-tiles over embed dim
MT = DL // P  # 4  m-tiles over local head dim (2 heads per m-tile)
SC = S // 512  # 4  512-wide chunks over sequence
SB = S // P  # 16 128-blocks over sequence
KT_D = DL // P  # 4  k-tiles over local head dim (proj contraction)
MT_E = E // P  # 8  m-tiles over embed dim (proj output)
VW = DH + 1  # 65: V columns per head + ones column
NGRP = 12  # stage-A matmul groups per chunk (4m x {q,k} + 4 V blocks)


# ------------------------------------------------- BIR multi-wait splitting
# The walrus build here accepts one sync-wait command per instruction; Tile
# attaches every outstanding dependency to the consumer. Split extras into
# single-wait EventSemaphore instructions just before the consumer (same
# engine => same blocking behavior).
_syncfix_done = [False]


def _install_syncfix():
    if _syncfix_done[0]:
        return
    _syncfix_done[0] = True
    import concourse.bass_utils as bu

    counter = [0]

    def split_multiwait(bir_json):
        d = json.loads(bir_json)
        changed = False
        for fn in d.get("functions", []):
            for bb in fn.get("blocks", []):
                new_insts = []
                for inst in bb.get("instructions", []):
                    si = inst.get("sync_info")
                    waits = (si or {}).get("on_wait") or []
                    if len(waits) > 1:
                        changed = True
                        for w in waits[:-1]:
                            counter[0] += 1
                            new_insts.append(
                                {
                                    "debug": inst.get("debug"),
                                    "engine": inst["engine"],
                                    "ins": [],
                                    "name": f"WSPLIT-{counter[0]}",
                                    "opcode": "EventSemaphore",
                                    "outs": [],
                                    "sync_info": {"on_update": [], "on_wait": [w]},
                                }
                            )
                        si["on_wait"] = [waits[-1]]
                    new_insts.append(inst)
                bb["instructions"] = new_insts
        if not changed:
            return bir_json if isinstance(bir_json, bytes) else bir_json.encode()
        return json.dumps(d).encode()

    orig = bu.compile_bir_kernel

    def patched(bir_json, tmpdir, neff_name="file.neff"):
        return orig(split_multiwait(bir_json), tmpdir, neff_name)

    bu.compile_bir_kernel = patched
    try:
        import concourse.bass2jax as b2j

        if hasattr(b2j, "compile_bir_kernel"):
            b2j.compile_bir_kernel = patched
    except ImportError:
        pass


# ------------------------------------------------------------ kernel build
def build_nc():
    import concourse.bass as bass
    import concourse.tile as tile
    from concourse import mybir

    f32 = mybir.dt.float32
    bf16 = mybir.dt.bfloat16
    EXP = mybir.ActivationFunctionType.Exp
    IDENT = mybir.ActivationFunctionType.Identity

    nc = bass.Bass()

    xt_ext = nc.dram_tensor("xt", [E, S], bf16, kind="ExternalInput")
    wq_ext = nc.dram_tensor("wq", [E, DL], bf16, kind="ExternalInput")
    wk_ext = nc.dram_tensor("wk", [E, DL], bf16, kind="ExternalInput")
    wv_ext = nc.dram_tensor("wv", [E, DL], bf16, kind="ExternalInput")
    wo_ext = nc.dram_tensor("wo", [DL, E], bf16, kind="ExternalInput")
    bo_ext = nc.dram_tensor("bo2", [E], f32, kind="ExternalInput")
    mask_ext = nc.dram_tensor("mask", [P, P], f32, kind="ExternalInput")
    sel64_ext = nc.dram_tensor("sel64", [DH, P], bf16, kind="ExternalInput")
    out_ext = nc.dram_tensor("outp", [E, S], bf16, kind="ExternalOutput")

    xt_r = xt_ext.rearrange("(kt p) s -> p kt s", p=P)
    wq_r = wq_ext.rearrange("(kt p) d -> p kt d", p=P)
    wk_r = wk_ext.rearrange("(kt p) d -> p kt d", p=P)
    wv_r = wv_ext.rearrange("(kt p) d -> p kt d", p=P)
    wo_r = wo_ext.rearrange("(kt p) e -> p kt e", p=P)
    bo_r = bo_ext.rearrange("(m p) -> p m", p=P)

    with tile.TileContext(nc) as tc:
        with tc.tile_pool(name="persist", bufs=1) as pers:
            # ---- persistent SBUF tensors
            qt = [pers.tile([P, S], bf16, tag=f"qt{m}", name=f"qt{m}") for m in range(MT)]
            kt = [pers.tile([P, S], bf16, tag=f"kt{m}", name=f"kt{m}") for m in range(MT)]
            v_sb = pers.tile([P, SB, HL * VW], bf16, tag="v")
            ct = [
                pers.tile([P, KT_D, 512], bf16, tag=f"ct{i}", name=f"ct{i}")
                for i in range(SC)
            ]
            wo_sb = pers.tile([P, KT_D, E], bf16, tag="wo")
            bo_sb = pers.tile([P, MT_E], f32, tag="bo")
            mask_sb = pers.tile([P, P], f32, tag="mask")
            mask2_sb = pers.tile([P, 2, P], f32, tag="mask2")
            sel64_sb = pers.tile([DH, P], bf16, tag="sel64")
            # reciprocal staging rows for the selector matmul: rows 0 and 32
            # (legal engine partition bases) of 8 ring slots; other rows stay
            # zero so the K=64 contraction only picks up rows 0/32
            recr64 = pers.tile([DH, 8, 512], bf16, tag="recr64")
            scr_i = pers.tile([1, DH], f32, tag="scr_i")
            scr_o = pers.tile([1, DH], f32, tag="scr_o")

            with (
                tc.tile_pool(name="wqkv", bufs=1) as wpool,
                tc.tile_pool(name="xt", bufs=4) as xpool,
                tc.tile_pool(name="probs", bufs=6) as ppool,
                tc.tile_pool(name="fin", bufs=4) as fpool,
                tc.tile_pool(name="ps_a", bufs=2, space="PSUM") as ps_a,
                tc.tile_pool(name="ps_s", bufs=2, space="PSUM") as ps_s,
                tc.tile_pool(name="ps_ctx", bufs=2, space="PSUM") as ps_ctx,
            ):
                wq_sb = wpool.tile([P, KT_E, DL], bf16, tag="wq")
                wk_sb = wpool.tile([P, KT_E, DL], bf16, tag="wk")
                wv_sb = wpool.tile([P, KT_E, DL], bf16, tag="wv")
                xc = [
                    xpool.tile([P, KT_E, 512], bf16, tag="xt", name=f"xt{c}")
                    for c in range(SC)
                ]
                # First-chunk operands split in k-slices across three queues
                # so the first matmul starts after ~256KB instead of ~3MB.
                for k2, ke in ((0, 1), (1, 3), (3, 5), (5, 8)):
                    nc.sync.dma_start(xc[0][:, k2:ke, :], xt_r[:, k2:ke, 0:512])
                    nc.gpsimd.dma_start(wq_sb[:, k2:ke, :], wq_r[:, k2:ke, :])
                    nc.scalar.dma_start(wk_sb[:, k2:ke, :], wk_r[:, k2:ke, :])
                for c in range(1, SC):
                    nc.sync.dma_start(
                        xc[c][:], xt_r[:, :, 512 * c : 512 * (c + 1)]
                    )
                nc.gpsimd.dma_start(wv_sb[:], wv_r)
                nc.gpsimd.dma_start(mask_sb[:], mask_ext[:, :])
                nc.gpsimd.dma_start(sel64_sb[:], sel64_ext[:, :])
                nc.gpsimd.dma_start(bo_sb[:], bo_r)
                nc.sync.dma_start(wo_sb[:], wo_r)

                # the fused row-sum ones column: initialize ALL of v_sb to
                # 1.0 up front (the V drains overwrite the 64 data columns
                # per head; column 64 stays 1.0). This replaces a 16K-
                # descriptor scatter DMA whose completion raced the first
                # AV matmuls once the stages were fused.
                nc.gpsimd.memset(v_sb[:], 1.0)
                # zero the selector rhs ring; preload the EXP table during
                # stage A (the lazy load otherwise hits the first scores)
                nc.gpsimd.memset(recr64[:], 0.0)
                nc.gpsimd.memset(scr_i[:], 0.0)
                nc.scalar.activation(scr_o[:], scr_i[:], EXP)
                for i in range(2):
                    nc.vector.tensor_copy(mask2_sb[:, i, :], mask_sb[:])

                # ---------------- stage A chunk = 12 matmul groups --------
                def a_group(c, gi):
                    if gi < 8:
                        m, w = divmod(gi, 2)
                        w_sb, dst = (wq_sb, qt) if w == 0 else (wk_sb, kt)
                        ps = ps_a.tile(
                            [P, 512], f32, tag="ps_a", name=f"psa{c}_{gi}"
                        )
                        for k in range(KT_E):
                            nc.tensor.matmul(
                                ps[:],
                                w_sb[:, k, P * m : P * (m + 1)],
                                xc[c][:, k, :],
                                start=(k == 0),
                                stop=(k == KT_E - 1),
                            )
                        nc.vector.tensor_copy(
                            dst[m][:, 512 * c : 512 * (c + 1)], ps[:]
                        )
                    else:
                        s = gi - 8
                        sb = 4 * c + s
                        ps = ps_a.tile([P, 512], f32, tag="ps_a", name=f"psv{sb}")
                        for k in range(KT_E):
                            nc.tensor.matmul(
                                ps[:],
                                xc[c][:, k, P * s : P * (s + 1)],
                                wv_sb[:, k, :],
                                start=(k == 0),
                                stop=(k == KT_E - 1),
                            )
                        for h in range(HL):
                            nc.vector.tensor_copy(
                                v_sb[:, sb, VW * h : VW * h + DH],
                                ps[:, DH * h : DH * (h + 1)],
                            )

                # filler state: next chunk's A-groups dripped in per block
                fill = {"c": None, "gi": 0, "acc": 0.0, "rate": 0.0}

                def set_filler(c_next, total_blocks):
                    fill["c"] = c_next
                    fill["gi"] = 0
                    fill["acc"] = 0.0
                    fill["rate"] = (
                        NGRP / total_blocks * 1.02 if c_next is not None else 0.0
                    )

                def filler():
                    if fill["c"] is None or fill["gi"] >= NGRP:
                        return
                    fill["acc"] += fill["rate"]
                    while fill["acc"] >= 1.0 and fill["gi"] < NGRP:
                        a_group(fill["c"], fill["gi"])
                        fill["gi"] += 1
                        fill["acc"] -= 1.0

                def flush_filler():
                    while fill["c"] is not None and fill["gi"] < NGRP:
                        a_group(fill["c"], fill["gi"])
                        fill["gi"] += 1

                # ---------------- softmax normalization helpers -----------
                def emit_recr(c, recs, pairs):
                    for t in pairs:
                        slot = (c % 2) * MT + t
                        for i, base in ((0, 0), (1, 32)):
                            h = 2 * t + i
                            src_ap = recs[
                                32 * (h % 4) : 32 * (h % 4) + 1,
                                512 * (h // 4) : 512 * (h // 4) + 512,
                            ]
                            dst_ap = recr64[base : base + 1, slot, :]
                            if i == 0:
                                nc.vector.tensor_copy(dst_ap, src_ap)
                            else:
                                nc.scalar.copy(dst_ap, src_ap)

                def emit_bc(c, pairs):
                    for t in pairs:
                        slot = (c % 2) * MT + t
                        bcp = ps_a.tile(
                            [P, 512], f32, tag="ps_a", name=f"bc{c}_{t}"
                        )
                        nc.tensor.matmul(
                            bcp[:],
                            sel64_sb[:, :],
                            recr64[:, slot, :],
                            start=True,
                            stop=True,
                        )
                        nc.vector.tensor_mul(
                            ct[c][:, t, :], ct[c][:, t, :], bcp[:]
                        )

                # ---------------- attention head pair ---------------------
                def b_pair(c, t, sums_sb):
                    heads = (2 * t, 2 * t + 1)
                    q_lo, q_hi = 512 * c, 512 * (c + 1)
                    nblk = 4 * c + 4
                    ctx_ps = {}
                    probs = {}
                    for h in heads:
                        ctx_ps[h] = ps_ctx.tile(
                            [VW, 512], f32, tag="ps_ctx", name=f"ctx_{c}_{h}"
                        )

                    def av_mm(h, j, first, last):
                        r = j - 4 * c
                        lo = P * r if r > 0 else 0
                        nc.tensor.matmul(
                            ctx_ps[h][:, lo:512],
                            v_sb[:, j, VW * h : VW * (h + 1)],
                            probs[j][:, h % 2, lo:512],
                            start=first,
                            stop=last,
                        )

                    for j in range(nblk):
                        r = j - 4 * c
                        slo = P * r if r > 0 else 0
                        ps = ps_s.tile(
                            [P, 2, 512], f32, tag="ps_s", name=f"pss_{c}_{t}_{j}"
                        )
                        for i, h in enumerate(heads):
                            hp = DH * i
                            nc.tensor.matmul(
                                ps[:, i, slo:512],
                                kt[t][hp : hp + DH, P * j : P * (j + 1)],
                                qt[t][hp : hp + DH, q_lo + slo : q_hi],
                                start=True,
                                stop=True,
                            )
                        if r >= 0:
                            nc.vector.tensor_add(
                                ps[:, :, P * r : P * (r + 1)],
                                ps[:, :, P * r : P * (r + 1)],
                                mask2_sb[:],
                            )
                        pr = ppool.tile(
                            [P, 2, 512], bf16, tag="probs", name=f"pr_{c}_{t}_{j}"
                        )
                        probs[j] = pr
                        nc.scalar.activation(
                            pr[:, :, slo:512], ps[:, :, slo:512], EXP, scale=SCALE
                        )
                        if j >= 1:
                            for h in heads:
                                av_mm(h, j - 1, first=(j == 1), last=False)
                        filler()
                    for h in heads:
                        av_mm(h, nblk - 1, first=(nblk == 1), last=True)

                    # stash unnormalized ctx + row sums (both vector; sums
                    # land at 32-aligned partitions x 2 col slots so one
                    # batched reciprocal serves all 8 heads)
                    for i, h in enumerate(heads):
                        hp = DH * i
                        nc.vector.tensor_copy(
                            ct[c][hp : hp + DH, t, :], ctx_ps[h][0:DH, :]
                        )
                        nc.vector.tensor_copy(
                            sums_sb[
                                32 * (h % 4) : 32 * (h % 4) + 1,
                                512 * (h // 4) : 512 * (h // 4) + 512,
                            ],
                            ctx_ps[h][DH : DH + 1, :],
                        )

                # ---------------- fused emission --------------------------
                for gi in range(NGRP):
                    a_group(0, gi)

                pending = None
                for c in range(SC):
                    sums_sb = fpool.tile(
                        [P, 1024], f32, tag="sums", name=f"sums{c}", bufs=2
                    )
                    recs = fpool.tile(
                        [P, 1024], f32, tag="recs", name=f"recs{c}", bufs=2
                    )
                    set_filler(c + 1 if c + 1 < SC else None, 4 * (4 * c + 4))
                    for t in range(MT):
                        b_pair(c, t, sums_sb)
                        if t == 1:
                            nc.vector.reciprocal(
                                recs[:, 0:512], sums_sb[:, 0:512]
                            )
                        if pending is not None:
                            if t == 0:
                                emit_recr(pending[0], pending[1], range(MT))
                            else:
                                emit_bc(pending[0], [t - 1])
                    nc.vector.reciprocal(recs[:, 512:1024], sums_sb[:, 512:1024])
                    flush_filler()
                    if pending is not None:
                        emit_bc(pending[0], [MT - 1])
                    pending = (c, recs)

                # final chunk: per-pair order so pairs 0/1 (half-0
                # reciprocal, long done) broadcast while half 1 finishes
                c_f, recs_f = pending
                for t in range(MT):
                    emit_recr(c_f, recs_f, [t])
                    emit_bc(c_f, [t])

            # ---- stage C: out^T = WoT-slice.T @ ctx^T (+ bias, group 0)
            with (
                tc.tile_pool(name="ostage", bufs=4) as opool,
                tc.tile_pool(name="ps_p", bufs=4, space="PSUM") as ps_p,
            ):
                qdma = [nc.sync, nc.scalar, nc.gpsimd]
                for m in range(MT_E):
                    for s in range(SC):
                        ps = ps_p.tile([P, 512], f32, tag="ps_p", name=f"psp{m}_{s}")
                        for k in range(KT_D):
                            nc.tensor.matmul(
                                ps[:],
                                wo_sb[:, k, P * m : P * (m + 1)],
                                ct[s][:, k, :],
                                start=(k == 0),
                                stop=(k == KT_D - 1),
                            )
                        ot = opool.tile(
                            [P, 512], bf16, tag="ostage", name=f"ot{m}_{s}"
                        )
                        nc.scalar.activation(
                            ot[:], ps[:], IDENT, bias=bo_sb[:, m : m + 1]
                        )
                        qdma[(m * SC + s) % 3].dma_start(
                            out_ext[P * m : P * (m + 1), 512 * s : 512 * (s + 1)],
                            ot[:],
                        )

    return nc


# ------------------------------------------------------------ PJRT runner
class _Runner:
    """Compile once, run many: mirrors bass2jax.run_bass_via_pjrt with a
    cached jitted executable."""

    def __init__(self, nc):
        import jax
        import jax.numpy  # noqa: F401
        from jax.sharding import Mesh, PartitionSpec
        from jax.experimental.shard_map import shard_map
        import concourse.bass2jax as b2j
        from concourse import mybir

        b2j.install_neuronx_cc_hook()
        self.jax = jax
        partition_name = (
            nc.partition_id_tensor.name if nc.partition_id_tensor else None
        )
        in_names = []
        out_names = []
        out_avals = []
        self.zero_shapes = []
        for alloc in nc.m.functions[0].allocations:
            if not isinstance(alloc, mybir.MemoryLocationSet):
                continue
            name = alloc.memorylocations[0].name
            if alloc.kind == "ExternalInput":
                if name == partition_name:
                    continue
                in_names.append(name)
            elif alloc.kind == "ExternalOutput":
                shape = tuple(alloc.tensor_shape)
                dtype = mybir.dt.np(alloc.dtype)
                out_names.append(name)
                out_avals.append(jax.core.ShapedArray(shape, dtype))
                self.zero_shapes.append((shape, dtype))
        self.in_names = in_names
        self.out_names = out_names
        self.out_avals = out_avals
        n_params = len(in_names)
        n_outs = len(out_avals)
        all_in = list(in_names) + list(out_names)
        if partition_name is not None:
            all_in.append(partition_name)

        def _body(*args):
            operands = list(args)
            if partition_name is not None:
                operands.append(b2j.partition_id_tensor())
            outs = b2j._bass_exec_p.bind(
                *operands,
                out_avals=tuple(out_avals),
                in_names=tuple(all_in),
                out_names=tuple(out_names),
                lowering_input_output_aliases=(),
                sim_require_finite=True,
                sim_require_nnan=True,
                nc=nc,
            )
            return tuple(outs)

        devices = jax.devices()[:NCORES]
        assert len(devices) == NCORES, f"need {NCORES} cores, got {len(devices)}"
        self.mesh = Mesh(np.asarray(devices), ("core",))
        in_specs = (PartitionSpec("core"),) * (n_params + n_outs)
        out_specs = (PartitionSpec("core"),) * n_outs
        self.fn = jax.jit(
            shard_map(
                _body,
                mesh=self.mesh,
                in_specs=in_specs,
                out_specs=out_specs,
                check_rep=False,
            ),
            donate_argnums=tuple(range(n_params, n_params + n_outs)),
            keep_unused=True,
        )

    def run(self, in_maps):
        concat_in = [
            np.concatenate([np.asarray(m[name]) for m in in_maps], axis=0)
            for name in self.in_names
        ]
        zeros = [
            np.zeros((NCORES * s[0], *s[1:]), dt) for s, dt in self.zero_shapes
        ]
        outs = self.fn(*concat_in, *zeros)
        return [
            {
                name: np.asarray(outs[i]).reshape(
                    NCORES, *self.out_avals[i].shape
                )[c]
                for i, name in enumerate(self.out_names)
            }
            for c in range(NCORES)
        ]


_cache = {}


def _get_runner():
    if "runner" not in _cache:
        _install_syncfix()
        _cache["runner"] = _Runner(build_nc())
    return _cache["runner"]


def make_in_maps(X, Wq, Wk, Wv, Wo, bo):
    import ml_dtypes

    bf16 = ml_dtypes.bfloat16
    X = np.asarray(X, dtype=np.float32)
    Wq = np.asarray(Wq, dtype=np.float32)
    Wk = np.asarray(Wk, dtype=np.float32)
    Wv = np.asarray(Wv, dtype=np.float32)
    Wo = np.asarray(Wo, dtype=np.float32)
    bo = np.asarray(bo, dtype=np.float32)

    kv = np.arange(P)[:, None]
    qq = np.arange(P)[None, :]
    mask = np.where(kv > qq, np.float32(NEG), np.float32(0.0))

    sel64 = np.zeros((DH, P), dtype=bf16)
    sel64[0, 0:DH] = 1
    sel64[32, DH:P] = 1

    in_maps = []
    for core in range(NCORES):
        b, g = divmod(core, 2)
        h0 = HL * g
        in_maps.append(
            {
                "xt": np.ascontiguousarray(X[b].T).astype(bf16),
                "wq": np.ascontiguousarray(
                    Wq[h0 : h0 + HL].transpose(1, 0, 2).reshape(E, DL)
                ).astype(bf16),
                "wk": np.ascontiguousarray(
                    Wk[h0 : h0 + HL].transpose(1, 0, 2).reshape(E, DL)
                ).astype(bf16),
                "wv": np.ascontiguousarray(
                    Wv[h0 : h0 + HL].transpose(1, 0, 2).reshape(E, DL)
                ).astype(bf16),
                "wo": np.ascontiguousarray(Wo[:, DL * g : DL * (g + 1)].T).astype(
                    bf16
                ),
                "bo2": bo if g == 0 else np.zeros_like(bo),
                "mask": mask,
                "sel64": sel64,
            }
        )
    return in_maps


def assemble(results):
    out = np.empty((B, S, E), dtype=np.float32)
    for b in range(B):
        acc = results[2 * b]["outp"].astype(np.float32) + results[
            2 * b + 1
        ]["outp"].astype(np.float32)
        out[b] = acc.T
    return out


def kernel(X, Wq, Wk, Wv, Wo, bo):
    runner = _get_runner()
    in_maps = make_in_maps(X, Wq, Wk, Wv, Wo, bo)
    results = runner.run(in_maps)
    return assemble(results)


# revision 27
# speedup vs baseline: 1.2528x; 1.0896x over previous
"""Multi-head causal attention (B=4, S=2048, E=1024, H=16, Dh=64) on 8
Trainium2 NeuronCores.

Sharding: data-parallel over the 4 batch elements x tensor-parallel over
heads (2 groups of 8). Core 2b+g handles batch b, heads 8g..8g+7. Each core
computes Q^T/K^T (head dim on partitions), V (natural layout, with a fused
ones-column so the attention-weight row sums fall out of the same matmul),
block-causal scores in transposed [kv, q] layout (no transposes needed:
softmax normalization is a reciprocal + selector-matmul partition
broadcast), the local-head context, and the output projection against its
slice of Wo. The two partial projections per batch are summed on the host
(the TP "all-reduce" of the sharding hint, done at gather time), which also
absorbs the out-transpose: the kernel emits out^T [E, S] in bf16.

All matmuls run in bf16 (fp32 accumulate): f32r streams at the same 1
cycle/row but draws enough power that the core throttles to a ~50%
utilization cap for half the runtime; bf16 also trims the r=3 diagonal
scores block to its live 128 columns (no N>=256 full-rate requirement).

KEY SCHEDULE IDEA (from per-engine totals of the unfused ancestors): the
QKV projections (stage A) are tensor-bound with an idle scalar engine,
while attention (stage B) is SCALAR-bound -- the EXP activations process
exactly the score elements at half the PE clock, so exp slightly exceeds
the PE's scores+AV work. Fusing the stages hides each bottleneck under the
other: QKV chunk c+1's matmul groups are interleaved at kv-block
granularity into attention chunk c's emission, giving the in-order tensor
queue filler work whenever the exp chain lags.

Other schedule notes:
- First-chunk X^T/Wq/Wk loads split per k-tile across three DMA queues
  (first matmul ~12us after launch, ~8us of that is the Tile preamble).
- Both heads of a pair share one [128, 2, 512] scores PSUM and one EXP.
- Softmax normalization pipelined one chunk behind: split exact DVE
  reciprocal (two 512-col halves, emitted mid-chunk), staging copies on
  vector+scalar, one K=64 selector-matmul broadcast + one ct multiply per
  head pair, consumers spread one-per-pair through the next chunk.
- All QKV psum drains on the vector engine: the scalar engine is reserved
  for EXP during attention.
- Output is bf16 and its DMA is spread over all three queues so the ~4MB
  store drains inside the projection instead of as a tail.
"""

import json
import os
import sys

for _p in ("/opt/trn_rl_repo",):
    if _p not in sys.path:
        sys.path.insert(0, _p)

import numpy as np

# ---------------------------------------------------------------- constants
B = 4
S = 2048
E = 1024
H = 16
DH = 64
HL = 8  # heads per core
DL = HL * DH  # 512, local head dim
P = 128
NCORES = 8
SCALE = 1.0 / 8.0  # 1/sqrt(DH)
NEG = -1.0e30

KT_E = E // P  # 8  k-tiles over embed dim
MT = DL // P  # 4  m-tiles over local head dim (2 heads per m-tile)
SC = S // 512  # 4  512-wide chunks over sequence
SB = S // P  # 16 128-blocks over sequence
KT_D = DL // P  # 4  k-tiles over local head dim (proj contraction)
MT_E = E // P  # 8  m-tiles over embed dim (proj output)
VW = DH + 1  # 65: V columns per head + ones column
NGRP = 12  # stage-A matmul groups per chunk (4m x {q,k} + 4 V blocks)


# ------------------------------------------------- BIR multi-wait splitting
# The walrus build here accepts one sync-wait command per instruction; Tile
# attaches every outstanding dependency to the consumer. Split extras into
# single-wait EventSemaphore instructions just before the consumer (same
# engine => same blocking behavior).
_syncfix_done = [False]


def _install_syncfix():
    if _syncfix_done[0]:
        return
    _syncfix_done[0] = True
    import concourse.bass_utils as bu

    counter = [0]

    def split_multiwait(bir_json):
        d = json.loads(bir_json)
        changed = False
        for fn in d.get("functions", []):
            for bb in fn.get("blocks", []):
                new_insts = []
                for inst in bb.get("instructions", []):
                    si = inst.get("sync_info")
                    waits = (si or {}).get("on_wait") or []
                    if len(waits) > 1:
                        changed = True
                        for w in waits[:-1]:
                            counter[0] += 1
                            new_insts.append(
                                {
                                    "debug": inst.get("debug"),
                                    "engine": inst["engine"],
                                    "ins": [],
                                    "name": f"WSPLIT-{counter[0]}",
                                    "opcode": "EventSemaphore",
                                    "outs": [],
                                    "sync_info": {"on_update": [], "on_wait": [w]},
                                }
                            )
                        si["on_wait"] = [waits[-1]]
                    new_insts.append(inst)
                bb["instructions"] = new_insts
        if not changed:
            return bir_json if isinstance(bir_json, bytes) else bir_json.encode()
        return json.dumps(d).encode()

    orig = bu.compile_bir_kernel

    def patched(bir_json, tmpdir, neff_name="file.neff"):
        return orig(split_multiwait(bir_json), tmpdir, neff_name)

    bu.compile_bir_kernel = patched
    try:
        import concourse.bass2jax as b2j

        if hasattr(b2j, "compile_bir_kernel"):
            b2j.compile_bir_kernel = patched
    except ImportError:
        pass


# ------------------------------------------------------------ kernel build
def build_nc():
    import concourse.bass as bass
    import concourse.tile as tile
    from concourse import mybir

    f32 = mybir.dt.float32
    bf16 = mybir.dt.bfloat16
    EXP = mybir.ActivationFunctionType.Exp
    IDENT = mybir.ActivationFunctionType.Identity

    nc = bass.Bass()

    xt_ext = nc.dram_tensor("xt", [E, S], bf16, kind="ExternalInput")
    wq_ext = nc.dram_tensor("wq", [E, DL], bf16, kind="ExternalInput")
    wk_ext = nc.dram_tensor("wk", [E, DL], bf16, kind="ExternalInput")
    wv_ext = nc.dram_tensor("wv", [E, DL], bf16, kind="ExternalInput")
    wo_ext = nc.dram_tensor("wo", [DL, E], bf16, kind="ExternalInput")
    mask_ext = nc.dram_tensor("mask", [P, P], f32, kind="ExternalInput")
    sel64_ext = nc.dram_tensor("sel64", [DH, P], bf16, kind="ExternalInput")
    out_ext = nc.dram_tensor("outp", [E, S], bf16, kind="ExternalOutput")

    xt_r = xt_ext.rearrange("(kt p) s -> p kt s", p=P)
    wq_r = wq_ext.rearrange("(kt p) d -> p kt d", p=P)
    wk_r = wk_ext.rearrange("(kt p) d -> p kt d", p=P)
    wv_r = wv_ext.rearrange("(kt p) d -> p kt d", p=P)
    wo_r = wo_ext.rearrange("(kt p) e -> p kt e", p=P)

    with tile.TileContext(nc) as tc:
        with tc.tile_pool(name="persist", bufs=1) as pers:
            # ---- persistent SBUF tensors
            qt = [pers.tile([P, S], bf16, tag=f"qt{m}", name=f"qt{m}") for m in range(MT)]
            kt = [pers.tile([P, S], bf16, tag=f"kt{m}", name=f"kt{m}") for m in range(MT)]
            v_sb = pers.tile([P, SB, HL * VW], bf16, tag="v")
            ct = [
                pers.tile([P, KT_D, 512], bf16, tag=f"ct{i}", name=f"ct{i}")
                for i in range(SC)
            ]
            wo_sb = pers.tile([P, KT_D, E], bf16, tag="wo")
            mask_sb = pers.tile([P, P], f32, tag="mask")
            mask2_sb = pers.tile([P, 2, P], f32, tag="mask2")
            sel64_sb = pers.tile([DH, P], bf16, tag="sel64")
            # reciprocal staging rows for the selector matmul: rows 0 and 32
            # (legal engine partition bases) of 8 ring slots; other rows stay
            # zero so the K=64 contraction only picks up rows 0/32
            recr64 = pers.tile([DH, 8, 512], bf16, tag="recr64")
            scr_i = pers.tile([1, DH], f32, tag="scr_i")
            scr_o = pers.tile([1, DH], f32, tag="scr_o")

            with (
                tc.tile_pool(name="wqkv", bufs=1) as wpool,
                tc.tile_pool(name="xt", bufs=4) as xpool,
                tc.tile_pool(name="probs", bufs=6) as ppool,
                tc.tile_pool(name="fin", bufs=4) as fpool,
                tc.tile_pool(name="ostage", bufs=4) as opool,
                tc.tile_pool(name="ps_a", bufs=2, space="PSUM") as ps_a,
                tc.tile_pool(name="ps_s", bufs=2, space="PSUM") as ps_s,
                tc.tile_pool(name="ps_ctx", bufs=2, space="PSUM") as ps_ctx,
            ):
                wq_sb = wpool.tile([P, KT_E, DL], bf16, tag="wq")
                wk_sb = wpool.tile([P, KT_E, DL], bf16, tag="wk")
                wv_sb = wpool.tile([P, KT_E, DL], bf16, tag="wv")
                xc = [
                    xpool.tile([P, KT_E, 512], bf16, tag="xt", name=f"xt{c}")
                    for c in range(SC)
                ]
                # First-chunk operands split in k-slices across three queues
                # so the first matmul starts after ~256KB instead of ~3MB.
                for k2, ke in ((0, 1), (1, 3), (3, 5), (5, 8)):
                    nc.sync.dma_start(xc[0][:, k2:ke, :], xt_r[:, k2:ke, 0:512])
                    nc.gpsimd.dma_start(wq_sb[:, k2:ke, :], wq_r[:, k2:ke, :])
                    nc.scalar.dma_start(wk_sb[:, k2:ke, :], wk_r[:, k2:ke, :])
                for c in range(1, SC):
                    nc.sync.dma_start(
                        xc[c][:], xt_r[:, :, 512 * c : 512 * (c + 1)]
                    )
                nc.gpsimd.dma_start(wv_sb[:], wv_r)
                nc.gpsimd.dma_start(mask_sb[:], mask_ext[:, :])
                nc.gpsimd.dma_start(sel64_sb[:], sel64_ext[:, :])
                nc.sync.dma_start(wo_sb[:], wo_r)

                # the fused row-sum ones column: initialize ALL of v_sb to
                # 1.0 up front (the V drains overwrite the 64 data columns
                # per head; column 64 stays 1.0). This replaces a 16K-
                # descriptor scatter DMA whose completion raced the first
                # AV matmuls once the stages were fused.
                nc.gpsimd.memset(v_sb[:], 1.0)
                # zero the selector rhs ring; preload the EXP table during
                # stage A (the lazy load otherwise hits the first scores)
                nc.gpsimd.memset(recr64[:], 0.0)
                nc.gpsimd.memset(scr_i[:], 0.0)
                nc.scalar.activation(scr_o[:], scr_i[:], EXP)
                for i in range(2):
                    nc.vector.tensor_copy(mask2_sb[:, i, :], mask_sb[:])

                # ---------------- stage A chunk = 12 matmul groups --------
                def a_group(c, gi):
                    if gi < 8:
                        m, w = divmod(gi, 2)
                        w_sb, dst = (wq_sb, qt) if w == 0 else (wk_sb, kt)
                        ps = ps_a.tile(
                            [P, 512], f32, tag="ps_a", name=f"psa{c}_{gi}"
                        )
                        for k in range(KT_E):
                            nc.tensor.matmul(
                                ps[:],
                                w_sb[:, k, P * m : P * (m + 1)],
                                xc[c][:, k, :],
                                start=(k == 0),
                                stop=(k == KT_E - 1),
                            )
                        nc.vector.tensor_copy(
                            dst[m][:, 512 * c : 512 * (c + 1)], ps[:]
                        )
                    else:
                        s = gi - 8
                        sb = 4 * c + s
                        ps = ps_a.tile([P, 512], f32, tag="ps_a", name=f"psv{sb}")
                        for k in range(KT_E):
                            nc.tensor.matmul(
                                ps[:],
                                xc[c][:, k, P * s : P * (s + 1)],
                                wv_sb[:, k, :],
                                start=(k == 0),
                                stop=(k == KT_E - 1),
                            )
                        for h in range(HL):
                            nc.vector.tensor_copy(
                                v_sb[:, sb, VW * h : VW * h + DH],
                                ps[:, DH * h : DH * (h + 1)],
                            )

                # projection group (stage C work, usable as chunk-3 filler
                # once a ct chunk's normalization multiplies are emitted)
                qdma = [nc.sync, nc.scalar, nc.gpsimd]
                pj_n = [0]

                def p_group(m, s):
                    ps = ps_a.tile([P, 512], f32, tag="ps_a", name=f"psp{m}_{s}")
                    for k in range(KT_D):
                        nc.tensor.matmul(
                            ps[:],
                            wo_sb[:, k, P * m : P * (m + 1)],
                            ct[s][:, k, :],
                            start=(k == 0),
                            stop=(k == KT_D - 1),
                        )
                    ot = opool.tile([P, 512], bf16, tag="ostage", name=f"ot{m}_{s}")
                    nc.vector.tensor_copy(ot[:], ps[:])
                    qdma[pj_n[0] % 3].dma_start(
                        out_ext[P * m : P * (m + 1), 512 * s : 512 * (s + 1)],
                        ot[:],
                    )
                    pj_n[0] += 1

                # filler: thunks dripped in per attention block so the
                # in-order PE queue has work whenever the exp chain lags
                fill = {"q": [], "acc": 0.0, "rate": 0.0}

                def set_filler(thunks, total_blocks):
                    fill["q"] = list(thunks)
                    fill["acc"] = 0.0
                    fill["rate"] = len(fill["q"]) / total_blocks * 1.02

                def filler():
                    if not fill["q"]:
                        return
                    fill["acc"] += fill["rate"]
                    while fill["acc"] >= 1.0 and fill["q"]:
                        fill["q"].pop(0)()
                        fill["acc"] -= 1.0

                def flush_filler():
                    while fill["q"]:
                        fill["q"].pop(0)()

                # ---------------- softmax normalization helpers -----------
                def emit_recr(c, recs, pairs):
                    for t in pairs:
                        slot = (c % 2) * MT + t
                        for i, base in ((0, 0), (1, 32)):
                            h = 2 * t + i
                            src_ap = recs[
                                32 * (h % 4) : 32 * (h % 4) + 1,
                                512 * (h // 4) : 512 * (h // 4) + 512,
                            ]
                            dst_ap = recr64[base : base + 1, slot, :]
                            if i == 0:
                                nc.vector.tensor_copy(dst_ap, src_ap)
                            else:
                                nc.scalar.copy(dst_ap, src_ap)

                def emit_bc(c, pairs):
                    for t in pairs:
                        slot = (c % 2) * MT + t
                        bcp = ps_a.tile(
                            [P, 512], f32, tag="ps_a", name=f"bc{c}_{t}"
                        )
                        nc.tensor.matmul(
                            bcp[:],
                            sel64_sb[:, :],
                            recr64[:, slot, :],
                            start=True,
                            stop=True,
                        )
                        nc.vector.tensor_mul(
                            ct[c][:, t, :], ct[c][:, t, :], bcp[:]
                        )

                # ---------------- attention head pair ---------------------
                def b_pair(c, t, sums_sb):
                    heads = (2 * t, 2 * t + 1)
                    q_lo, q_hi = 512 * c, 512 * (c + 1)
                    nblk = 4 * c + 4
                    ctx_ps = {}
                    probs = {}
                    for h in heads:
                        ctx_ps[h] = ps_ctx.tile(
                            [VW, 512], f32, tag="ps_ctx", name=f"ctx_{c}_{h}"
                        )

                    def av_mm(h, j, first, last):
                        r = j - 4 * c
                        lo = P * r if r > 0 else 0
                        nc.tensor.matmul(
                            ctx_ps[h][:, lo:512],
                            v_sb[:, j, VW * h : VW * (h + 1)],
                            probs[j][:, h % 2, lo:512],
                            start=first,
                            stop=last,
                        )

                    for j in range(nblk):
                        r = j - 4 * c
                        slo = P * r if r > 0 else 0
                        ps = ps_s.tile(
                            [P, 2, 512], f32, tag="ps_s", name=f"pss_{c}_{t}_{j}"
                        )
                        for i, h in enumerate(heads):
                            hp = DH * i
                            nc.tensor.matmul(
                                ps[:, i, slo:512],
                                kt[t][hp : hp + DH, P * j : P * (j + 1)],
                                qt[t][hp : hp + DH, q_lo + slo : q_hi],
                                start=True,
                                stop=True,
                            )
                        if r >= 0:
                            nc.vector.tensor_add(
                                ps[:, :, P * r : P * (r + 1)],
                                ps[:, :, P * r : P * (r + 1)],
                                mask2_sb[:],
                            )
                        pr = ppool.tile(
                            [P, 2, 512], bf16, tag="probs", name=f"pr_{c}_{t}_{j}"
                        )
                        probs[j] = pr
                        nc.scalar.activation(
                            pr[:, :, slo:512], ps[:, :, slo:512], EXP, scale=SCALE
                        )
                        if j >= 1:
                            for h in heads:
                                av_mm(h, j - 1, first=(j == 1), last=False)
                        filler()
                    for h in heads:
                        av_mm(h, nblk - 1, first=(nblk == 1), last=True)

                    # stash unnormalized ctx + row sums (both vector; sums
                    # land at 32-aligned partitions x 2 col slots so one
                    # batched reciprocal serves all 8 heads)
                    for i, h in enumerate(heads):
                        hp = DH * i
                        nc.vector.tensor_copy(
                            ct[c][hp : hp + DH, t, :], ctx_ps[h][0:DH, :]
                        )
                        nc.vector.tensor_copy(
                            sums_sb[
                                32 * (h % 4) : 32 * (h % 4) + 1,
                                512 * (h // 4) : 512 * (h // 4) + 512,
                            ],
                            ctx_ps[h][DH : DH + 1, :],
                        )

                # ---------------- fused emission --------------------------
                for gi in range(NGRP):
                    a_group(0, gi)

                a_thunks = {
                    c: [
                        (lambda c_=c, gi_=gi: a_group(c_, gi_))
                        for gi in range(NGRP)
                    ]
                    for c in (1, 2, 3)
                }
                # chunk-3 filler: projection groups whose ct chunks are
                # normalized by then (s=0 during chunk 1, s=1 during chunk 2)
                p01 = [
                    (lambda m_=m, s_=s: p_group(m_, s_))
                    for s in range(2)
                    for m in range(MT_E)
                ]
                seg_fill = {0: a_thunks[1], 1: a_thunks[2], 2: a_thunks[3], 3: p01}

                pending = None
                for c in range(SC):
                    sums_sb = fpool.tile(
                        [P, 1024], f32, tag="sums", name=f"sums{c}", bufs=2
                    )
                    recs = fpool.tile(
                        [P, 1024], f32, tag="recs", name=f"recs{c}", bufs=2
                    )
                    set_filler(seg_fill[c], 4 * (4 * c + 4))
                    for t in range(MT):
                        b_pair(c, t, sums_sb)
                        if t == 1:
                            nc.vector.reciprocal(
                                recs[:, 0:512], sums_sb[:, 0:512]
                            )
                        if pending is not None:
                            if t == 0:
                                emit_recr(pending[0], pending[1], range(MT))
                            elif c == SC - 1:
                                # consolidate chunk 2's broadcasts at t==1 so
                                # the s=2 projection groups are legal below
                                emit_bc(pending[0], range(MT))
                                pending = None
                            else:
                                emit_bc(pending[0], [t - 1])
                        if c == SC - 1 and t >= 2:
                            for m in range(4 * (t - 2), 4 * (t - 1)):
                                p_group(m, 2)
                    nc.vector.reciprocal(recs[:, 512:1024], sums_sb[:, 512:1024])
                    flush_filler()
                    if pending is not None:
                        emit_bc(pending[0], [MT - 1])
                    pending = (c, recs)

                # tail: per-pair normalization of the last chunk (pairs 0/1
                # use the half-0 reciprocal, long done), then the s=3
                # projection groups (k ascending hits pairs 2/3 last)
                c_f, recs_f = pending
                for t in range(MT):
                    emit_recr(c_f, recs_f, [t])
                    emit_bc(c_f, [t])
                for m in range(MT_E):
                    p_group(m, SC - 1)

    return nc


# ------------------------------------------------------------ PJRT runner
class _Runner:
    """Compile once, run many: mirrors bass2jax.run_bass_via_pjrt with a
    cached jitted executable."""

    def __init__(self, nc):
        import jax
        import jax.numpy  # noqa: F401
        from jax.sharding import Mesh, PartitionSpec
        from jax.experimental.shard_map import shard_map
        import concourse.bass2jax as b2j
        from concourse import mybir

        b2j.install_neuronx_cc_hook()
        self.jax = jax
        partition_name = (
            nc.partition_id_tensor.name if nc.partition_id_tensor else None
        )
        in_names = []
        out_names = []
        out_avals = []
        self.zero_shapes = []
        for alloc in nc.m.functions[0].allocations:
            if not isinstance(alloc, mybir.MemoryLocationSet):
                continue
            name = alloc.memorylocations[0].name
            if alloc.kind == "ExternalInput":
                if name == partition_name:
                    continue
                in_names.append(name)
            elif alloc.kind == "ExternalOutput":
                shape = tuple(alloc.tensor_shape)
                dtype = mybir.dt.np(alloc.dtype)
                out_names.append(name)
                out_avals.append(jax.core.ShapedArray(shape, dtype))
                self.zero_shapes.append((shape, dtype))
        self.in_names = in_names
        self.out_names = out_names
        self.out_avals = out_avals
        n_params = len(in_names)
        n_outs = len(out_avals)
        all_in = list(in_names) + list(out_names)
        if partition_name is not None:
            all_in.append(partition_name)

        def _body(*args):
            operands = list(args)
            if partition_name is not None:
                operands.append(b2j.partition_id_tensor())
            outs = b2j._bass_exec_p.bind(
                *operands,
                out_avals=tuple(out_avals),
                in_names=tuple(all_in),
                out_names=tuple(out_names),
                lowering_input_output_aliases=(),
                sim_require_finite=True,
                sim_require_nnan=True,
                nc=nc,
            )
            return tuple(outs)

        devices = jax.devices()[:NCORES]
        assert len(devices) == NCORES, f"need {NCORES} cores, got {len(devices)}"
        self.mesh = Mesh(np.asarray(devices), ("core",))
        in_specs = (PartitionSpec("core"),) * (n_params + n_outs)
        out_specs = (PartitionSpec("core"),) * n_outs
        self.fn = jax.jit(
            shard_map(
                _body,
                mesh=self.mesh,
                in_specs=in_specs,
                out_specs=out_specs,
                check_rep=False,
            ),
            donate_argnums=tuple(range(n_params, n_params + n_outs)),
            keep_unused=True,
        )

    def run(self, in_maps):
        concat_in = [
            np.concatenate([np.asarray(m[name]) for m in in_maps], axis=0)
            for name in self.in_names
        ]
        zeros = [
            np.zeros((NCORES * s[0], *s[1:]), dt) for s, dt in self.zero_shapes
        ]
        outs = self.fn(*concat_in, *zeros)
        return [
            {
                name: np.asarray(outs[i]).reshape(
                    NCORES, *self.out_avals[i].shape
                )[c]
                for i, name in enumerate(self.out_names)
            }
            for c in range(NCORES)
        ]


_cache = {}


def _get_runner():
    if "runner" not in _cache:
        _install_syncfix()
        _cache["runner"] = _Runner(build_nc())
    return _cache["runner"]


def make_in_maps(X, Wq, Wk, Wv, Wo, bo):
    import ml_dtypes

    bf16 = ml_dtypes.bfloat16
    X = np.asarray(X, dtype=np.float32)
    Wq = np.asarray(Wq, dtype=np.float32)
    Wk = np.asarray(Wk, dtype=np.float32)
    Wv = np.asarray(Wv, dtype=np.float32)
    Wo = np.asarray(Wo, dtype=np.float32)
    bo = np.asarray(bo, dtype=np.float32)

    kv = np.arange(P)[:, None]
    qq = np.arange(P)[None, :]
    mask = np.where(kv > qq, np.float32(NEG), np.float32(0.0))

    sel64 = np.zeros((DH, P), dtype=bf16)
    sel64[0, 0:DH] = 1
    sel64[32, DH:P] = 1

    in_maps = []
    for core in range(NCORES):
        b, g = divmod(core, 2)
        h0 = HL * g
        in_maps.append(
            {
                "xt": np.ascontiguousarray(X[b].T).astype(bf16),
                "wq": np.ascontiguousarray(
                    Wq[h0 : h0 + HL].transpose(1, 0, 2).reshape(E, DL)
                ).astype(bf16),
                "wk": np.ascontiguousarray(
                    Wk[h0 : h0 + HL].transpose(1, 0, 2).reshape(E, DL)
                ).astype(bf16),
                "wv": np.ascontiguousarray(
                    Wv[h0 : h0 + HL].transpose(1, 0, 2).reshape(E, DL)
                ).astype(bf16),
                "wo": np.ascontiguousarray(Wo[:, DL * g : DL * (g + 1)].T).astype(
                    bf16
                ),
                "mask": mask,
                "sel64": sel64,
            }
        )
    return in_maps


def assemble(results, bo):
    bo = np.asarray(bo, dtype=np.float32)
    out = np.empty((B, S, E), dtype=np.float32)
    for b in range(B):
        acc = results[2 * b]["outp"].astype(np.float32) + results[
            2 * b + 1
        ]["outp"].astype(np.float32)
        out[b] = acc.T + bo
    return out


def kernel(X, Wq, Wk, Wv, Wo, bo):
    runner = _get_runner()
    in_maps = make_in_maps(X, Wq, Wk, Wv, Wo, bo)
    results = runner.run(in_maps)
    return assemble(results, bo)
